# revision 6
# baseline (speedup 1.0000x reference)
"""Trainium2 Bass kernel for a 7-layer Riptide-style binarized CNN.

Strategy (data-parallel over 8 NeuronCores, 64 images/core):
  - conv1 (full precision) is computed as one K=27 fp32 matmul per 450
    output positions, from a host-built im2col matrix [27, 64*900].
  - Every BN(+relu)(+maxpool)->sign boundary is folded into a per-output-
    channel threshold: the next layer's +-1 input is Sign(psum + bias),
    computed by the scalar engine directly out of PSUM (bias = -T, or +BIG
    when the channel is always +1).  maxpool commutes with relu and with
    the monotone BN, so pooling is done on raw PSUM integers with two
    strided tensor_max ops before the Sign.
  - conv2..7 operands are +-1 encoded as fp8e4m3; the matmul accumulates
    exactly in fp32 PSUM (integer counts), so the binary convs are exact.
  - Padding pads zeros *before* sign, so pad regions are sign(0)=+1: the
    padded activation buffers are memset to +1 once and only the interior
    is rewritten.
  - Images stream through the net in groups of 32 so all activations fit
    in SBUF; weights are resident.
"""

import os
import sys

sys.path.insert(0, "/opt/trn_rl_repo")

import numpy as np
import ml_dtypes
from contextlib import ExitStack

import concourse.bass as bass  # noqa: F401  (bass types used indirectly)
import concourse.mybir as mybir
import concourse.tile as tile
from concourse import bacc
from concourse.bass_utils import run_bass_kernel_spmd
from concourse.masks import make_identity

F32 = mybir.dt.float32
FP8 = mybir.dt.float8e4
NP8 = ml_dtypes.float8_e4m3fn

NCORES = 8
B = 512
NB = B // NCORES  # images per core
EPS = 1e-3
BIG = 1e30

TAPS9 = [(dy, dx) for dy in range(3) for dx in range(3)]

# layer -> (KC, MC): contraction chunks of 128, output-channel chunks of 128
KC = {2: 1, 3: 1, 4: 2, 5: 2, 6: 4}
MC = {2: 1, 3: 2, 4: 2, 5: 4, 6: 4}

# weight offsets inside the packed fp8 weight tile (free-dim elements)
_OFF = {}
_o = 0
for _l in (2, 3, 4, 5, 6):
    _OFF[_l] = _o
    _o += 9 * KC[_l] * MC[_l] * 128
_OFF[7] = _o
WTOT = _o + 16 * 4 * 10

# threshold-vector columns inside cvec [128, 14]
CVCOL = {1: 0, 2: 1, 3: 2, 4: 4, 5: 6, 6: 10}

_prog_cache = {}


def _woff(layer, tap, kc, mc):
    return _OFF[layer] + ((tap * KC[layer] + kc) * MC[layer] + mc) * 128


def build_program(nb=NB, g=32):
    assert nb % g == 0
    c4 = min(4, g)   # L1 dma chunk, images
    c2 = min(2, g)   # L3/L4 chunk, images
    c8 = min(8, g)   # L5/L6 chunk, images
    assert g % c4 == 0 and g % c8 == 0

    nc = bacc.Bacc("TRN2", target_bir_lowering=False, debug=False)
    Sign = mybir.ActivationFunctionType.Sign
    Exp = mybir.ActivationFunctionType.Exp
    Identity = mybir.ActivationFunctionType.Identity

    x1 = nc.declare_dram_parameter("x1", [27, nb * 900], F32, isOutput=False)
    w1 = nc.declare_dram_parameter("w1", [27, 128], F32, isOutput=False)
    wall = nc.declare_dram_parameter("wall", [128, WTOT], FP8, isOutput=False)
    cvec = nc.declare_dram_parameter("cvec", [128, 14], F32, isOutput=False)
    bn7 = nc.declare_dram_parameter("bn7", [10, 2], F32, isOutput=False)
    y = nc.declare_dram_parameter("y", [nb, 10], F32, isOutput=True)

    with tile.TileContext(nc) as tc, ExitStack() as ctx:
        consts = ctx.enter_context(tc.tile_pool(name="consts", bufs=1))
        sbufs = ctx.enter_context(tc.tile_pool(name="sbufs", bufs=1))
        xpool = ctx.enter_context(tc.tile_pool(name="xpool", bufs=3))
        post = ctx.enter_context(tc.tile_pool(name="post", bufs=4))
        psum = ctx.enter_context(tc.tile_pool(name="psum", bufs=4, space="PSUM"))
        psum7 = ctx.enter_context(tc.tile_pool(name="psum7", bufs=1, space="PSUM"))

        w1sb = consts.tile([27, 128], F32)
        nc.sync.dma_start(out=w1sb, in_=w1[:, :])
        wsb = consts.tile([128, WTOT], FP8)
        nc.sync.dma_start(out=wsb, in_=wall[:, :])
        cv = consts.tile([128, 14], F32)
        nc.sync.dma_start(out=cv, in_=cvec[:, :])
        bn7sb = consts.tile([10, 2], F32)
        nc.sync.dma_start(out=bn7sb, in_=bn7[:, :])
        ident = consts.tile([10, 10], F32)
        make_identity(nc, ident)

        # persistent activation buffers (one group's worth, reused)
        s2 = sbufs.tile([128, g, 34, 34], FP8)
        s3 = sbufs.tile([128, g, 18, 18], FP8)
        s4 = sbufs.tile([128, 2, g, 18, 18], FP8)
        s5 = sbufs.tile([128, 2, g, 10, 10], FP8)
        s6 = sbufs.tile([128, 4, g, 10, 10], FP8)
        s7 = sbufs.tile([128, 4, g, 4, 4], FP8)
        for t in (s2, s3, s4, s5, s6):
            nc.gpsimd.memset(t, 1.0)

        def w8(layer, tap, kc, mc):
            o = _woff(layer, tap, kc, mc)
            return wsb[:, o : o + 128]

        def tbias(layer, mc):
            c = CVCOL[layer] + mc
            return cv[:, c : c + 1]

        for grp in range(nb // g):
            i00 = grp * g

            # ---------------- L1: conv1 (fp32) + fused bias/relu/BN1/pad/sign
            for ch in range(g // c4):
                xt = xpool.tile([27, c4 * 900], F32, tag="xt")
                base = (i00 + ch * c4) * 900
                nc.sync.dma_start(out=xt, in_=x1[:, base : base + c4 * 900])
                for sc in range(2 * c4):
                    p = psum.tile([128, 15, 30], F32, tag="ps")
                    nc.tensor.matmul(
                        p, w1sb, xt[:, sc * 450 : (sc + 1) * 450],
                        start=True, stop=True,
                    )
                    img = ch * c4 + sc // 2
                    r0 = (sc % 2) * 15
                    nc.scalar.activation(
                        s2[:, img, 2 + r0 : 17 + r0, 2:32], p, Sign,
                        bias=tbias(1, 0), scale=1.0,
                    )

            # ---------------- L2: binconv 128->128, pool, BN2, pad, sign
            for img in range(g):
                for rc in range(2):
                    p = psum.tile([128, 16, 32], F32, tag="ps")
                    for t, (dy, dx) in enumerate(TAPS9):
                        nc.tensor.matmul(
                            p, w8(2, t, 0, 0),
                            s2[:, img, rc * 16 + dy : rc * 16 + dy + 16, dx : dx + 32],
                            start=(t == 0), stop=(t == 8),
                        )
                    t1 = post.tile([128, 16, 16], F32, tag="t1")
                    nc.vector.reduce_max(
                        t1, p.rearrange("p y (x two) -> p y x two", two=2),
                        axis=mybir.AxisListType.X,
                    )
                    t2 = post.tile([128, 8, 16], F32, tag="t2")
                    nc.vector.reduce_max(
                        t2, t1.rearrange("p (y two) x -> p y x two", two=2),
                        axis=mybir.AxisListType.X,
                    )
                    nc.scalar.activation(
                        s3[:, img, 1 + rc * 8 : 9 + rc * 8, 1:17], t2, Sign,
                        bias=tbias(2, 0), scale=1.0,
                    )

            # ---------------- L3: binconv 128->256, BN3, pad, sign
            for chk in range(g // c2):
                i0 = chk * c2
                for mc in range(2):
                    p = psum.tile([128, c2, 16, 16], F32, tag="ps")
                    for t, (dy, dx) in enumerate(TAPS9):
                        nc.tensor.matmul(
                            p, w8(3, t, 0, mc),
                            s3[:, i0 : i0 + c2, dy : dy + 16, dx : dx + 16],
                            start=(t == 0), stop=(t == 8),
                        )
                    nc.scalar.activation(
                        s4[:, mc, i0 : i0 + c2, 1:17, 1:17], p, Sign,
                        bias=tbias(3, mc), scale=1.0,
                    )

            # ---------------- L4: binconv 256->256, pool, BN4, pad, sign
            for chk in range(g // c2):
                i0 = chk * c2
                for mc in range(2):
                    p = psum.tile([128, c2, 16, 16], F32, tag="ps")
                    k = 0
                    for kc in range(2):
                        for t, (dy, dx) in enumerate(TAPS9):
                            nc.tensor.matmul(
                                p, w8(4, t, kc, mc),
                                s4[:, kc, i0 : i0 + c2, dy : dy + 16, dx : dx + 16],
                                start=(k == 0), stop=(k == 17),
                            )
                            k += 1
                    t1 = post.tile([128, c2, 16, 8], F32, tag="t1")
                    nc.vector.reduce_max(
                        t1, p.rearrange("p i y (x two) -> p i y x two", two=2),
                        axis=mybir.AxisListType.X,
                    )
                    t2 = post.tile([128, c2, 8, 8], F32, tag="t2")
                    nc.vector.reduce_max(
                        t2, t1.rearrange("p i (y two) x -> p i y x two", two=2),
                        axis=mybir.AxisListType.X,
                    )
                    nc.scalar.activation(
                        s5[:, mc, i0 : i0 + c2, 1:9, 1:9], t2, Sign,
                        bias=tbias(4, mc), scale=1.0,
                    )

            # ---------------- L5: binconv 256->512, BN5, pad, sign
            for chk in range(g // c8):
                i0 = chk * c8
                for mc in range(4):
                    p = psum.tile([128, c8, 8, 8], F32, tag="ps")
                    k = 0
                    for kc in range(2):
                        for t, (dy, dx) in enumerate(TAPS9):
                            nc.tensor.matmul(
                                p, w8(5, t, kc, mc),
                                s5[:, kc, i0 : i0 + c8, dy : dy + 8, dx : dx + 8],
                                start=(k == 0), stop=(k == 17),
                            )
                            k += 1
                    nc.scalar.activation(
                        s6[:, mc, i0 : i0 + c8, 1:9, 1:9], p, Sign,
                        bias=tbias(5, mc), scale=1.0,
                    )

            # ---------------- L6: binconv 512->512, pool, BN6, sign (no pad)
            for chk in range(g // c8):
                i0 = chk * c8
                for mc in range(4):
                    p = psum.tile([128, c8, 8, 8], F32, tag="ps")
                    k = 0
                    for kc in range(4):
                        for t, (dy, dx) in enumerate(TAPS9):
                            nc.tensor.matmul(
                                p, w8(6, t, kc, mc),
                                s6[:, kc, i0 : i0 + c8, dy : dy + 8, dx : dx + 8],
                                start=(k == 0), stop=(k == 35),
                            )
                            k += 1
                    t1 = post.tile([128, c8, 8, 4], F32, tag="t1")
                    nc.vector.reduce_max(
                        t1, p.rearrange("p i y (x two) -> p i y x two", two=2),
                        axis=mybir.AxisListType.X,
                    )
                    t2 = post.tile([128, c8, 4, 4], F32, tag="t2")
                    nc.vector.reduce_max(
                        t2, t1.rearrange("p i (y two) x -> p i y x two", two=2),
                        axis=mybir.AxisListType.X,
                    )
                    nc.scalar.activation(
                        s7[:, mc, i0 : i0 + c8, :, :], t2, Sign,
                        bias=tbias(6, mc), scale=1.0,
                    )

            # ---------------- L7: binconv 512->10 (4x4), relu, BN7, softmax
            s7v = s7.rearrange("p k i y x -> p k i (y x)")
            p7 = psum7.tile([10, g], F32, tag="p7")
            k = 0
            for t in range(16):
                for kc in range(4):
                    o = _OFF[7] + (t * 4 + kc) * 10
                    nc.tensor.matmul(
                        p7, wsb[:, o : o + 10], s7v[:, kc, :, t],
                        start=(k == 0), stop=(k == 63),
                    )
                    k += 1
            h7 = post.tile([10, g], F32, tag="h7")
            nc.vector.tensor_scalar_max(h7, p7, 0.0)
            v7 = post.tile([10, g], F32, tag="v7")
            nc.scalar.activation(
                v7, h7, Identity, bias=bn7sb[:, 1:2], scale=bn7sb[:, 0:1]
            )
            pt = psum7.tile([g, 10], F32, tag="pt")
            nc.tensor.transpose(pt, v7, ident)
            mx = post.tile([g, 1], F32, tag="mx")
            nc.vector.reduce_max(mx, pt, axis=mybir.AxisListType.X)
            nmx = post.tile([g, 1], F32, tag="nmx")
            nc.vector.tensor_scalar_mul(nmx, mx, -1.0)
            ex = post.tile([g, 10], F32, tag="ex")
            nc.scalar.activation(ex, pt, Exp, bias=nmx, scale=1.0)
            sm = post.tile([g, 1], F32, tag="sm")
            nc.vector.reduce_sum(sm, ex, axis=mybir.AxisListType.X)
            ri = post.tile([g, 1], F32, tag="ri")
            nc.vector.reciprocal(ri, sm)
            yo = post.tile([g, 10], F32, tag="yo")
            nc.vector.tensor_scalar_mul(yo, ex, ri)
            nc.sync.dma_start(out=y[i00 : i00 + g, :], in_=yo)

    nc.compile()
    return nc


# ------------------------------------------------------------------ host prep

def _thresh_bias(gm, be, m, v):
    """bias such that next-layer input = Sign(pre_bn_value + bias).

    pre_bn u >= T  <=>  g*(relu-or-pool(u) - m)*rsqrt(v+eps) + be >= 0,
    given g > 0.  When the channel is always +1, bias = +BIG.
    """
    a = gm.astype(np.float64) / np.sqrt(v.astype(np.float64) + EPS)
    c = be.astype(np.float64) - a * m.astype(np.float64)
    return np.where(c < 0.0, c / a, BIG).astype(np.float32)  # -T = c/a


def _pack_w(wl):
    """sign(w) [3,3,Cin,Cout] -> [128, 9*KC*MC*128] fp8, (tap,kc,mc,q) order."""
    s = np.where(wl >= 0, 1.0, -1.0).astype(np.float32)
    _, _, cin, cout = wl.shape
    kc, mcn = cin // 128, cout // 128
    a = s.reshape(3, 3, kc, 128, mcn, 128)
    a = np.ascontiguousarray(a.transpose(3, 0, 1, 2, 4, 5))
    return a.reshape(128, 9 * kc * mcn * 128).astype(NP8)


def _prep_shared(inputs):
    d = {k: np.asarray(v, np.float32) for k, v in inputs.items()}

    wall = np.empty((128, WTOT), dtype=NP8)
    for layer in (2, 3, 4, 5, 6):
        wl = _pack_w(d[f"w{layer}"])
        wall[:, _OFF[layer] : _OFF[layer] + wl.shape[1]] = wl
    s7w = np.where(d["w7"] >= 0, 1.0, -1.0).astype(np.float32)
    a = s7w.reshape(4, 4, 4, 128, 10).transpose(3, 0, 1, 2, 4)
    wall[:, _OFF[7] :] = np.ascontiguousarray(a).reshape(128, 640).astype(NP8)

    cvec = np.zeros((128, 14), dtype=np.float32)
    tb1 = _thresh_bias(d["g1"], d["be1"], d["m1"], d["v1"])
    cvec[:, 0] = (d["b1"].astype(np.float64) + tb1.astype(np.float64)).astype(
        np.float32
    )
    for layer in (2, 3, 4, 5, 6):
        tb = _thresh_bias(
            d[f"g{layer}"], d[f"be{layer}"], d[f"m{layer}"], d[f"v{layer}"]
        )
        cvec[:, CVCOL[layer] : CVCOL[layer] + MC[layer]] = tb.reshape(
            MC[layer], 128
        ).T

    a7 = (d["g7"].astype(np.float64) / np.sqrt(d["v7"].astype(np.float64) + EPS))
    c7 = d["be7"].astype(np.float64) - a7 * d["m7"].astype(np.float64)
    bn7 = np.stack(
        [a7.astype(np.float32), c7.astype(np.float32)], axis=1
    )  # [10, 2]

    w1r = np.ascontiguousarray(d["w1"].reshape(27, 128))
    return d, wall, cvec, bn7, w1r


def _im2col(x):
    """x [B,32,32,3] -> [27, B, 900] f32, row order (dy,dx,c)."""
    from numpy.lib.stride_tricks import sliding_window_view

    sw = sliding_window_view(x, (3, 3), axis=(1, 2))  # [B,30,30,3,3,3] b,y,x,c,dy,dx
    im = sw.transpose(4, 5, 3, 0, 1, 2).reshape(27, x.shape[0], 900)
    return np.ascontiguousarray(im)


LAST_RESULTS = None


def kernel(**inputs):
    global LAST_RESULTS
    nb, g = NB, 32
    key = (nb, g)
    if key not in _prog_cache:
        _prog_cache[key] = build_program(nb, g)
    nc = _prog_cache[key]

    d, wall, cvec, bn7, w1r = _prep_shared(inputs)
    im = _im2col(d["x"])  # [27, 512, 900]

    in_maps = []
    for c in range(NCORES):
        xi = np.ascontiguousarray(
            im[:, c * nb : (c + 1) * nb, :]
        ).reshape(27, nb * 900)
        in_maps.append(
            {"x1": xi, "w1": w1r, "wall": wall, "cvec": cvec, "bn7": bn7}
        )

    trace = bool(int(os.environ.get("KERNEL_TRACE", "0")))
    res = run_bass_kernel_spmd(
        nc, in_maps, core_ids=list(range(NCORES)), trace=trace
    )
    LAST_RESULTS = res
    out = np.concatenate([res.results[i]["y"] for i in range(NCORES)], axis=0)
    return out.astype(np.float32)


# revision 9
# speedup vs baseline: 1.1771x; 1.1771x over previous
"""Trainium2 Bass kernel for a 7-layer Riptide-style binarized CNN.

Strategy (data-parallel over 8 NeuronCores, 64 images/core):
  - conv1 (full precision) is one K=27 float32r matmul per 450 output
    positions, from a host-built im2col matrix [27, 64*900].
  - Every BN(+relu)(+maxpool)->sign boundary folds into a per-output-
    channel threshold: next layer's +-1 input is Sign(psum + bias) on the
    scalar engine straight out of PSUM (bias = -T, or +BIG for always-+1
    channels).  maxpool commutes with relu and monotone BN, so pooling
    runs on raw PSUM integers (two strided reduce_max ops) before Sign.
  - conv2..7 operands are +-1 fp8e4m3; PSUM accumulates exact integer
    counts in fp32, so the binary convs are exact.
  - Zero-padding happens before sign, so pad regions are sign(0)=+1: the
    padded activation buffers are memset to +1 once; only interiors are
    rewritten.
  - L4/L5/L6 (Cin>=256) use fp8 DoubleRow: contraction-pairs (c, c+128)
    live in one partition as [128, 2, span] flat layouts; the conv runs
    on the full padded grid (garbage at right/bottom edges discarded by
    the strided post-op reads), so the moving AP is a single contiguous
    span and each matmul does 2 taps-worth of MACs per cycle.
  - Images stream in groups of 32; weights are resident in SBUF.
"""

import os
import sys

sys.path.insert(0, "/opt/trn_rl_repo")

import numpy as np
import ml_dtypes
from contextlib import ExitStack

import concourse.bass as bass  # noqa: F401
import concourse.mybir as mybir
import concourse.tile as tile
from concourse import bacc
from concourse.bass_utils import run_bass_kernel_spmd
from concourse.masks import make_identity

F32 = mybir.dt.float32
F32R = mybir.dt.float32r
FP8 = mybir.dt.float8e4
NP8 = ml_dtypes.float8_e4m3fn
DR = mybir.MatmulPerfMode.DoubleRow

NCORES = 8
B = 512
NB = B // NCORES
EPS = 1e-3
BIG = 1e30

TAPS9 = [(dy, dx) for dy in range(3) for dx in range(3)]

KC = {2: 1, 3: 1, 4: 2, 5: 2, 6: 4}
MC = {2: 1, 3: 2, 4: 2, 5: 4, 6: 4}

_OFF = {}
_o = 0
for _l in (2, 3, 4, 5, 6):
    _OFF[_l] = _o
    _o += 9 * KC[_l] * MC[_l] * 128
_OFF[7] = _o
WTOT = _o + 16 * 4 * 10

CVCOL = {1: 0, 2: 1, 3: 2, 4: 4, 5: 6, 6: 10}

_prog_cache = {}


def _woff(layer, tap, kc, mc):
    return _OFF[layer] + ((tap * KC[layer] + kc) * MC[layer] + mc) * 128


def build_program(nb=NB, g=32):
    assert nb % g == 0
    c4 = min(4, g)   # L1 dma chunk, images
    c2 = min(2, g)   # L3 chunk, images
    cL = min(4, g)   # L5/L6 chunk, images (full-grid span 4*100=400)
    assert g % c4 == 0 and g % cL == 0

    span4 = g * 324 + 48   # flat padded span per kc block (16B aligned)
    span5 = g * 100 + 32
    span6 = g * 100 + 32

    nc = bacc.Bacc("TRN2", target_bir_lowering=False, debug=False)
    Sign = mybir.ActivationFunctionType.Sign
    Exp = mybir.ActivationFunctionType.Exp
    Identity = mybir.ActivationFunctionType.Identity
    AX = mybir.AxisListType.X

    x1 = nc.declare_dram_parameter("x1", [27, nb * 900], F32, isOutput=False)
    w1 = nc.declare_dram_parameter("w1", [27, 128], F32, isOutput=False)
    wall = nc.declare_dram_parameter("wall", [128, WTOT], FP8, isOutput=False)
    cvec = nc.declare_dram_parameter("cvec", [128, 14], F32, isOutput=False)
    bn7 = nc.declare_dram_parameter("bn7", [10, 2], F32, isOutput=False)
    y = nc.declare_dram_parameter("y", [nb, 10], F32, isOutput=True)

    with tile.TileContext(nc) as tc, ExitStack() as ctx:
        consts = ctx.enter_context(tc.tile_pool(name="consts", bufs=1))
        sbufs = ctx.enter_context(tc.tile_pool(name="sbufs", bufs=1))
        xpool = ctx.enter_context(tc.tile_pool(name="xpool", bufs=3))
        post = ctx.enter_context(tc.tile_pool(name="post", bufs=4))
        psum = ctx.enter_context(tc.tile_pool(name="psum", bufs=4, space="PSUM"))
        psum7 = ctx.enter_context(tc.tile_pool(name="psum7", bufs=1, space="PSUM"))

        w1sb = consts.tile([27, 128], F32)
        nc.sync.dma_start(out=w1sb, in_=w1[:, :])
        wsb = consts.tile([128, WTOT], FP8)
        nc.sync.dma_start(out=wsb, in_=wall[:, :])
        cv = consts.tile([128, 14], F32)
        nc.sync.dma_start(out=cv, in_=cvec[:, :])
        bn7sb = consts.tile([10, 2], F32)
        nc.sync.dma_start(out=bn7sb, in_=bn7[:, :])
        ident = consts.tile([10, 10], F32)
        make_identity(nc, ident)

        # DoubleRow weight views: [128, (tap), (kc), (mc), 128]
        def wview(layer):
            n = 9 * KC[layer] * MC[layer] * 128
            return wsb[:, _OFF[layer] : _OFF[layer] + n].rearrange(
                "p (t k m q) -> p t k m q",
                t=9, k=KC[layer], m=MC[layer], q=128,
            )

        wl4, wl5, wl6 = wview(4), wview(5), wview(6)

        # persistent activation buffers (one group's worth, reused)
        s2 = sbufs.tile([128, g, 34, 34], FP8)
        s3 = sbufs.tile([128, g, 18, 18], FP8)
        s4f = sbufs.tile([128, 2, span4], FP8)
        s5f = sbufs.tile([128, 2, span5], FP8)
        s6f = sbufs.tile([128, 4, span6], FP8)
        s7 = sbufs.tile([128, 4, g, 4, 4], FP8)
        s4i = s4f[:, :, : g * 324].rearrange(
            "p k (i y x) -> p k i y x", i=g, y=18, x=18
        )
        s5i = s5f[:, :, : g * 100].rearrange(
            "p k (i y x) -> p k i y x", i=g, y=10, x=10
        )
        s6i = s6f[:, :, : g * 100].rearrange(
            "p k (i y x) -> p k i y x", i=g, y=10, x=10
        )
        for t in (s2, s3, s4f, s5f, s6f):
            nc.gpsimd.memset(t, 1.0)

        def w8(layer, tap, kc, mc):
            o = _woff(layer, tap, kc, mc)
            return wsb[:, o : o + 128]

        def tbias(layer, mc):
            c = CVCOL[layer] + mc
            return cv[:, c : c + 1]

        for grp in range(nb // g):
            i00 = grp * g

            # ------------- L1: conv1 (f32r) + fused bias/relu/BN1/pad/sign
            for ch in range(g // c4):
                xt = xpool.tile([27, c4 * 900], F32, tag="xt")
                base = (i00 + ch * c4) * 900
                nc.sync.dma_start(out=xt, in_=x1[:, base : base + c4 * 900])
                for sc in range(2 * c4):
                    p = psum.tile([128, 15, 30], F32, tag="ps")
                    nc.tensor.matmul(
                        p, w1sb,
                        xt[:, sc * 450 : (sc + 1) * 450],
                        start=True, stop=True,
                    )
                    img = ch * c4 + sc // 2
                    r0 = (sc % 2) * 15
                    nc.scalar.activation(
                        s2[:, img, 2 + r0 : 17 + r0, 2:32], p, Sign,
                        bias=tbias(1, 0), scale=1.0,
                    )

            # ------------- L2: binconv 128->128, pool, BN2, pad, sign
            for img in range(g):
                for rc in range(2):
                    p = psum.tile([128, 16, 32], F32, tag="ps")
                    for t, (dy, dx) in enumerate(TAPS9):
                        nc.tensor.matmul(
                            p, w8(2, t, 0, 0),
                            s2[:, img, rc * 16 + dy : rc * 16 + dy + 16, dx : dx + 32],
                            start=(t == 0), stop=(t == 8),
                        )
                    t1 = post.tile([128, 16, 16], F32, tag="t1")
                    nc.vector.reduce_max(
                        t1, p.rearrange("p y (x two) -> p y x two", two=2), axis=AX
                    )
                    t2 = post.tile([128, 8, 16], F32, tag="t2")
                    nc.vector.reduce_max(
                        t2, t1.rearrange("p (y two) x -> p y x two", two=2), axis=AX
                    )
                    nc.scalar.activation(
                        s3[:, img, 1 + rc * 8 : 9 + rc * 8, 1:17], t2, Sign,
                        bias=tbias(2, 0), scale=1.0,
                    )

            # ------------- L3: binconv 128->256, BN3, pad, sign
            for chk in range(g // c2):
                i0 = chk * c2
                for mc in range(2):
                    p = psum.tile([128, c2, 16, 16], F32, tag="ps")
                    for t, (dy, dx) in enumerate(TAPS9):
                        nc.tensor.matmul(
                            p, w8(3, t, 0, mc),
                            s3[:, i0 : i0 + c2, dy : dy + 16, dx : dx + 16],
                            start=(t == 0), stop=(t == 8),
                        )
                    nc.scalar.activation(
                        s4i[:, mc, i0 : i0 + c2, 1:17, 1:17], p, Sign,
                        bias=tbias(3, mc), scale=1.0,
                    )

            # ------------- L4: binconv 256->256 (DoubleRow), pool, BN4, sign
            for img in range(g):
                for mc in range(2):
                    p = psum.tile([128, 324], F32, tag="ps")
                    for t, (dy, dx) in enumerate(TAPS9):
                        o = img * 324 + dy * 18 + dx
                        nc.tensor.matmul(
                            p, wl4[:, t, 0:2, mc, :], s4f[:, :, o : o + 324],
                            start=(t == 0), stop=(t == 8), perf_mode=DR,
                        )
                    pv = p.rearrange("p (y x) -> p y x", y=18, x=18)
                    t1 = post.tile([128, 16, 8], F32, tag="t1")
                    nc.vector.reduce_max(
                        t1,
                        pv[:, 0:16, 0:16].rearrange(
                            "p y (x two) -> p y x two", two=2
                        ),
                        axis=AX,
                    )
                    t2 = post.tile([128, 8, 8], F32, tag="t2")
                    nc.vector.reduce_max(
                        t2, t1.rearrange("p (y two) x -> p y x two", two=2), axis=AX
                    )
                    nc.scalar.activation(
                        s5i[:, mc, img, 1:9, 1:9], t2, Sign,
                        bias=tbias(4, mc), scale=1.0,
                    )

            # ------------- L5: binconv 256->512 (DoubleRow), BN5, pad, sign
            for chk in range(g // cL):
                i0 = chk * cL
                for mc in range(4):
                    p = psum.tile([128, cL * 100], F32, tag="ps")
                    for t, (dy, dx) in enumerate(TAPS9):
                        o = i0 * 100 + dy * 10 + dx
                        nc.tensor.matmul(
                            p, wl5[:, t, 0:2, mc, :],
                            s5f[:, :, o : o + cL * 100],
                            start=(t == 0), stop=(t == 8), perf_mode=DR,
                        )
                    pv = p.rearrange("p (i y x) -> p i y x", i=cL, y=10, x=10)
                    nc.scalar.activation(
                        s6i[:, mc, i0 : i0 + cL, 1:9, 1:9], pv[:, :, 0:8, 0:8],
                        Sign, bias=tbias(5, mc), scale=1.0,
                    )

            # ------------- L6: binconv 512->512 (DoubleRow), pool, BN6, sign
            for chk in range(g // cL):
                i0 = chk * cL
                for mc in range(4):
                    p = psum.tile([128, cL * 100], F32, tag="ps")
                    k = 0
                    for kp in range(2):
                        for t, (dy, dx) in enumerate(TAPS9):
                            o = i0 * 100 + dy * 10 + dx
                            nc.tensor.matmul(
                                p, wl6[:, t, 2 * kp : 2 * kp + 2, mc, :],
                                s6f[:, 2 * kp : 2 * kp + 2, o : o + cL * 100],
                                start=(k == 0), stop=(k == 17), perf_mode=DR,
                            )
                            k += 1
                    pv = p.rearrange("p (i y x) -> p i y x", i=cL, y=10, x=10)
                    t1 = post.tile([128, cL, 8, 4], F32, tag="t1")
                    nc.vector.reduce_max(
                        t1,
                        pv[:, :, 0:8, 0:8].rearrange(
                            "p i y (x two) -> p i y x two", two=2
                        ),
                        axis=AX,
                    )
                    t2 = post.tile([128, cL, 4, 4], F32, tag="t2")
                    nc.vector.reduce_max(
                        t2, t1.rearrange("p i (y two) x -> p i y x two", two=2),
                        axis=AX,
                    )
                    nc.scalar.activation(
                        s7[:, mc, i0 : i0 + cL, :, :], t2, Sign,
                        bias=tbias(6, mc), scale=1.0,
                    )

            # ------------- L7: binconv 512->10 (4x4), relu, BN7, softmax
            s7v = s7.rearrange("p k i y x -> p k i (y x)")
            p7 = psum7.tile([10, g], F32, tag="p7")
            k = 0
            for t in range(16):
                for kc in range(4):
                    o = _OFF[7] + (t * 4 + kc) * 10
                    nc.tensor.matmul(
                        p7, wsb[:, o : o + 10], s7v[:, kc, :, t],
                        start=(k == 0), stop=(k == 63),
                    )
                    k += 1
            h7 = post.tile([10, g], F32, tag="h7")
            nc.vector.tensor_scalar_max(h7, p7, 0.0)
            v7 = post.tile([10, g], F32, tag="v7")
            nc.scalar.activation(
                v7, h7, Identity, bias=bn7sb[:, 1:2], scale=bn7sb[:, 0:1]
            )
            pt = psum7.tile([g, 10], F32, tag="pt")
            nc.tensor.transpose(pt, v7, ident)
            mx = post.tile([g, 1], F32, tag="mx")
            nc.vector.reduce_max(mx, pt, axis=AX)
            nmx = post.tile([g, 1], F32, tag="nmx")
            nc.vector.tensor_scalar_mul(nmx, mx, -1.0)
            ex = post.tile([g, 10], F32, tag="ex")
            nc.scalar.activation(ex, pt, Exp, bias=nmx, scale=1.0)
            sm = post.tile([g, 1], F32, tag="sm")
            nc.vector.reduce_sum(sm, ex, axis=AX)
            ri = post.tile([g, 1], F32, tag="ri")
            nc.vector.reciprocal(ri, sm)
            yo = post.tile([g, 10], F32, tag="yo")
            nc.vector.tensor_scalar_mul(yo, ex, ri)
            nc.sync.dma_start(out=y[i00 : i00 + g, :], in_=yo)

    nc.compile()
    return nc


# ------------------------------------------------------------------ host prep

def _thresh_bias(gm, be, m, v):
    """bias such that next-layer input = Sign(pre_bn_value + bias)."""
    a = gm.astype(np.float64) / np.sqrt(v.astype(np.float64) + EPS)
    c = be.astype(np.float64) - a * m.astype(np.float64)
    return np.where(c < 0.0, c / a, BIG).astype(np.float32)  # -T = c/a


def _pack_w(wl):
    """sign(w) [3,3,Cin,Cout] -> [128, 9*KC*MC*128] fp8, (tap,kc,mc,q) order."""
    s = np.where(wl >= 0, 1.0, -1.0).astype(np.float32)
    _, _, cin, cout = wl.shape
    kc, mcn = cin // 128, cout // 128
    a = s.reshape(3, 3, kc, 128, mcn, 128)
    a = np.ascontiguousarray(a.transpose(3, 0, 1, 2, 4, 5))
    return a.reshape(128, 9 * kc * mcn * 128).astype(NP8)


def _prep_shared(inputs):
    d = {k: np.asarray(v, np.float32) for k, v in inputs.items()}

    wall = np.empty((128, WTOT), dtype=NP8)
    for layer in (2, 3, 4, 5, 6):
        wl = _pack_w(d[f"w{layer}"])
        wall[:, _OFF[layer] : _OFF[layer] + wl.shape[1]] = wl
    s7w = np.where(d["w7"] >= 0, 1.0, -1.0).astype(np.float32)
    a = s7w.reshape(4, 4, 4, 128, 10).transpose(3, 0, 1, 2, 4)
    wall[:, _OFF[7] :] = np.ascontiguousarray(a).reshape(128, 640).astype(NP8)

    cvec = np.zeros((128, 14), dtype=np.float32)
    tb1 = _thresh_bias(d["g1"], d["be1"], d["m1"], d["v1"])
    cvec[:, 0] = (d["b1"].astype(np.float64) + tb1.astype(np.float64)).astype(
        np.float32
    )
    for layer in (2, 3, 4, 5, 6):
        tb = _thresh_bias(
            d[f"g{layer}"], d[f"be{layer}"], d[f"m{layer}"], d[f"v{layer}"]
        )
        cvec[:, CVCOL[layer] : CVCOL[layer] + MC[layer]] = tb.reshape(
            MC[layer], 128
        ).T

    a7 = d["g7"].astype(np.float64) / np.sqrt(d["v7"].astype(np.float64) + EPS)
    c7 = d["be7"].astype(np.float64) - a7 * d["m7"].astype(np.float64)
    bn7 = np.stack([a7.astype(np.float32), c7.astype(np.float32)], axis=1)

    w1r = np.ascontiguousarray(d["w1"].reshape(27, 128))
    return d, wall, cvec, bn7, w1r


def _im2col(x):
    """x [B,32,32,3] -> [27, B, 900] f32, row order (dy,dx,c)."""
    from numpy.lib.stride_tricks import sliding_window_view

    sw = sliding_window_view(x, (3, 3), axis=(1, 2))  # [B,30,30,3,3,3]
    im = sw.transpose(4, 5, 3, 0, 1, 2).reshape(27, x.shape[0], 900)
    return np.ascontiguousarray(im)


LAST_RESULTS = None


def kernel(**inputs):
    global LAST_RESULTS
    nb, g = NB, 32
    key = (nb, g)
    if key not in _prog_cache:
        _prog_cache[key] = build_program(nb, g)
    nc = _prog_cache[key]

    d, wall, cvec, bn7, w1r = _prep_shared(inputs)
    im = _im2col(d["x"])

    in_maps = []
    for c in range(NCORES):
        xi = np.ascontiguousarray(im[:, c * nb : (c + 1) * nb, :]).reshape(
            27, nb * 900
        )
        in_maps.append(
            {"x1": xi, "w1": w1r, "wall": wall, "cvec": cvec, "bn7": bn7}
        )

    trace = bool(int(os.environ.get("KERNEL_TRACE", "0")))
    res = run_bass_kernel_spmd(
        nc, in_maps, core_ids=list(range(NCORES)), trace=trace
    )
    LAST_RESULTS = res
    out = np.concatenate([res.results[i]["y"] for i in range(NCORES)], axis=0)
    return out.astype(np.float32)


# revision 11
# speedup vs baseline: 1.1789x; 1.0016x over previous
"""Trainium2 Bass kernel for a 7-layer Riptide-style binarized CNN.

Strategy (data-parallel over 8 NeuronCores, 64 images/core):
  - conv1 (full precision) is one K=27 float32r matmul per 450 output
    positions, from a host-built im2col matrix [27, 64*900].
  - Every BN(+relu)(+maxpool)->sign boundary folds into a per-output-
    channel threshold: next layer's +-1 input is Sign(psum + bias) on the
    scalar engine straight out of PSUM (bias = -T, or +BIG for always-+1
    channels).  maxpool commutes with relu and monotone BN, so pooling
    runs on raw PSUM integers (two strided reduce_max ops) before Sign.
  - conv2..7 operands are +-1 fp8e4m3; PSUM accumulates exact integer
    counts in fp32, so the binary convs are exact.
  - Zero-padding happens before sign, so pad regions are sign(0)=+1: the
    padded activation buffers are memset to +1 once; only interiors are
    rewritten.
  - L4/L5/L6 (Cin>=256) use fp8 DoubleRow: contraction-pairs (c, c+128)
    live in one partition as [128, 2, span] flat layouts; the conv runs
    on the full padded grid (garbage at right/bottom edges discarded by
    the strided post-op reads), so the moving AP is a single contiguous
    span and each matmul does 2 taps-worth of MACs per cycle.
  - Images stream in groups of 32; weights are resident in SBUF.
"""

import os
import sys

sys.path.insert(0, "/opt/trn_rl_repo")

import numpy as np
import ml_dtypes
from contextlib import ExitStack

import concourse.bass as bass  # noqa: F401
import concourse.mybir as mybir
import concourse.tile as tile
from concourse import bacc
from concourse.bass_utils import run_bass_kernel_spmd
from concourse.masks import make_identity

F32 = mybir.dt.float32
F32R = mybir.dt.float32r
FP8 = mybir.dt.float8e4
NP8 = ml_dtypes.float8_e4m3fn
DR = mybir.MatmulPerfMode.DoubleRow

NCORES = 8
B = 512
NB = B // NCORES
EPS = 1e-3
BIG = 1e30

TAPS9 = [(dy, dx) for dy in range(3) for dx in range(3)]

KC = {2: 1, 3: 1, 4: 2, 5: 2, 6: 4}
MC = {2: 1, 3: 2, 4: 2, 5: 4, 6: 4}

_OFF = {}
_o = 0
for _l in (2, 3, 4, 5, 6):
    _OFF[_l] = _o
    _o += 9 * KC[_l] * MC[_l] * 128
_OFF[7] = _o
WTOT = _o + 16 * 4 * 10

CVCOL = {1: 0, 2: 1, 3: 2, 4: 4, 5: 6, 6: 10}

_prog_cache = {}


def _woff(layer, tap, kc, mc):
    return _OFF[layer] + ((tap * KC[layer] + kc) * MC[layer] + mc) * 128


def build_program(nb=NB, g=32):
    assert nb % g == 0
    c4 = min(4, g)   # L1 dma chunk, images
    c2 = min(2, g)   # L3 chunk, images
    cL = min(4, g)   # L5/L6 chunk, images (full-grid span 4*100=400)
    assert g % c4 == 0 and g % cL == 0

    span4 = g * 324 + 48   # flat padded span per kc block (16B aligned)
    span5 = g * 100 + 32
    span6 = g * 100 + 32

    nc = bacc.Bacc("TRN2", target_bir_lowering=False, debug=False)
    Sign = mybir.ActivationFunctionType.Sign
    Exp = mybir.ActivationFunctionType.Exp
    Identity = mybir.ActivationFunctionType.Identity
    AX = mybir.AxisListType.X

    x1 = nc.declare_dram_parameter("x1", [27, nb * 900], F32, isOutput=False)
    w1 = nc.declare_dram_parameter("w1", [27, 128], F32, isOutput=False)
    wall = nc.declare_dram_parameter("wall", [128, WTOT], FP8, isOutput=False)
    cvec = nc.declare_dram_parameter("cvec", [128, 14], F32, isOutput=False)
    bn7 = nc.declare_dram_parameter("bn7", [10, 2], F32, isOutput=False)
    y = nc.declare_dram_parameter("y", [nb, 10], F32, isOutput=True)

    with tile.TileContext(nc) as tc, ExitStack() as ctx:
        consts = ctx.enter_context(tc.tile_pool(name="consts", bufs=1))
        sbufs = ctx.enter_context(tc.tile_pool(name="sbufs", bufs=1))
        xpool = ctx.enter_context(tc.tile_pool(name="xpool", bufs=3))
        post = ctx.enter_context(tc.tile_pool(name="post", bufs=4))
        psum = ctx.enter_context(tc.tile_pool(name="psum", bufs=2, space="PSUM"))
        psum7 = ctx.enter_context(tc.tile_pool(name="psum7", bufs=1, space="PSUM"))

        w1sb = consts.tile([27, 128], F32)
        nc.sync.dma_start(out=w1sb, in_=w1[:, :])
        wsb = consts.tile([128, WTOT], FP8)
        nc.sync.dma_start(out=wsb, in_=wall[:, :])
        cv = consts.tile([128, 14], F32)
        nc.sync.dma_start(out=cv, in_=cvec[:, :])
        bn7sb = consts.tile([10, 2], F32)
        nc.sync.dma_start(out=bn7sb, in_=bn7[:, :])
        ident = consts.tile([10, 10], F32)
        make_identity(nc, ident)

        # DoubleRow weight views: [128, (tap), (kc), (mc), 128]
        def wview(layer):
            n = 9 * KC[layer] * MC[layer] * 128
            return wsb[:, _OFF[layer] : _OFF[layer] + n].rearrange(
                "p (t k m q) -> p t k m q",
                t=9, k=KC[layer], m=MC[layer], q=128,
            )

        wl4, wl5, wl6 = wview(4), wview(5), wview(6)

        # persistent activation buffers (one group's worth, reused)
        s2 = sbufs.tile([128, g, 34, 34], FP8)
        s3 = sbufs.tile([128, g, 18, 18], FP8)
        s4f = sbufs.tile([128, 2, span4], FP8)
        s5f = sbufs.tile([128, 2, span5], FP8)
        s6f = sbufs.tile([128, 4, span6], FP8)
        s7 = sbufs.tile([128, 4, g, 4, 4], FP8)
        s4i = s4f[:, :, : g * 324].rearrange(
            "p k (i y x) -> p k i y x", i=g, y=18, x=18
        )
        s5i = s5f[:, :, : g * 100].rearrange(
            "p k (i y x) -> p k i y x", i=g, y=10, x=10
        )
        s6i = s6f[:, :, : g * 100].rearrange(
            "p k (i y x) -> p k i y x", i=g, y=10, x=10
        )
        for t in (s2, s3, s4f, s5f, s6f):
            nc.gpsimd.memset(t, 1.0)

        def w8(layer, tap, kc, mc):
            o = _woff(layer, tap, kc, mc)
            return wsb[:, o : o + 128]

        def tbias(layer, mc):
            c = CVCOL[layer] + mc
            return cv[:, c : c + 1]

        for grp in range(nb // g):
            i00 = grp * g

            # ------------- L1: conv1 (f32r) + fused bias/relu/BN1/pad/sign
            for ch in range(g // c4):
                xt = xpool.tile([27, c4 * 900], F32, tag="xt")
                base = (i00 + ch * c4) * 900
                nc.sync.dma_start(out=xt, in_=x1[:, base : base + c4 * 900])
                for sc in range(2 * c4):
                    p = psum.tile([128, 15, 30], F32, tag=f"ps{sc % 3}")
                    nc.tensor.matmul(
                        p, w1sb,
                        xt[:, sc * 450 : (sc + 1) * 450],
                        start=True, stop=True,
                    )
                    img = ch * c4 + sc // 2
                    r0 = (sc % 2) * 15
                    nc.scalar.activation(
                        s2[:, img, 2 + r0 : 17 + r0, 2:32], p, Sign,
                        bias=tbias(1, 0), scale=1.0,
                    )

            # ------------- L2: binconv 128->128, pool, BN2, pad, sign
            for img in range(g):
                for rc in range(2):
                    p = psum.tile([128, 16, 32], F32, tag=f"ps{(2 * img + rc) % 3}")
                    for t, (dy, dx) in enumerate(TAPS9):
                        nc.tensor.matmul(
                            p, w8(2, t, 0, 0),
                            s2[:, img, rc * 16 + dy : rc * 16 + dy + 16, dx : dx + 32],
                            start=(t == 0), stop=(t == 8),
                        )
                    t1 = post.tile([128, 16, 16], F32, tag="t1")
                    nc.vector.reduce_max(
                        t1, p.rearrange("p y (x two) -> p y x two", two=2), axis=AX
                    )
                    t2 = post.tile([128, 8, 16], F32, tag="t2")
                    nc.vector.reduce_max(
                        t2, t1.rearrange("p (y two) x -> p y x two", two=2), axis=AX
                    )
                    nc.scalar.activation(
                        s3[:, img, 1 + rc * 8 : 9 + rc * 8, 1:17], t2, Sign,
                        bias=tbias(2, 0), scale=1.0,
                    )

            # ------------- L3: binconv 128->256, BN3, pad, sign
            for chk in range(g // c2):
                i0 = chk * c2
                for mc in range(2):
                    p = psum.tile([128, c2, 16, 16], F32, tag=f"ps{(2 * chk + mc) % 3}")
                    for t, (dy, dx) in enumerate(TAPS9):
                        nc.tensor.matmul(
                            p, w8(3, t, 0, mc),
                            s3[:, i0 : i0 + c2, dy : dy + 16, dx : dx + 16],
                            start=(t == 0), stop=(t == 8),
                        )
                    nc.scalar.activation(
                        s4i[:, mc, i0 : i0 + c2, 1:17, 1:17], p, Sign,
                        bias=tbias(3, mc), scale=1.0,
                    )

            # ------------- L4: binconv 256->256 (DoubleRow), pool, BN4, sign
            # weight-reuse: each (tap, mc) weight load feeds 3 psum accumulators
            for mc in range(2):
                for b0 in range(0, g, 3):
                    bs = min(3, g - b0)
                    pss = [
                        psum.tile([128, 324], F32, tag=f"ps{j}", name=f"ps{j}")
                        for j in range(bs)
                    ]
                    for t, (dy, dx) in enumerate(TAPS9):
                        for j in range(bs):
                            o = (b0 + j) * 324 + dy * 18 + dx
                            nc.tensor.matmul(
                                pss[j], wl4[:, t, 0:2, mc, :],
                                s4f[:, :, o : o + 324],
                                start=(t == 0), stop=(t == 8), perf_mode=DR,
                            )
                    for j in range(bs):
                        pv = pss[j].rearrange("p (y x) -> p y x", y=18, x=18)
                        t1 = post.tile([128, 16, 8], F32, tag="t1")
                        nc.vector.reduce_max(
                            t1,
                            pv[:, 0:16, 0:16].rearrange(
                                "p y (x two) -> p y x two", two=2
                            ),
                            axis=AX,
                        )
                        t2 = post.tile([128, 8, 8], F32, tag="t2")
                        nc.vector.reduce_max(
                            t2, t1.rearrange("p (y two) x -> p y x two", two=2),
                            axis=AX,
                        )
                        nc.scalar.activation(
                            s5i[:, mc, b0 + j, 1:9, 1:9], t2, Sign,
                            bias=tbias(4, mc), scale=1.0,
                        )

            # ------------- L5: binconv 256->512 (DoubleRow), BN5, pad, sign
            nchk = g // cL
            for mc in range(4):
                for cb in range(0, nchk, 3):
                    bs = min(3, nchk - cb)
                    pss = [
                        psum.tile([128, cL * 100], F32, tag=f"ps{j}", name=f"ps{j}")
                        for j in range(bs)
                    ]
                    for t, (dy, dx) in enumerate(TAPS9):
                        for j in range(bs):
                            o = (cb + j) * cL * 100 + dy * 10 + dx
                            nc.tensor.matmul(
                                pss[j], wl5[:, t, 0:2, mc, :],
                                s5f[:, :, o : o + cL * 100],
                                start=(t == 0), stop=(t == 8), perf_mode=DR,
                            )
                    for j in range(bs):
                        i0 = (cb + j) * cL
                        pv = pss[j].rearrange(
                            "p (i y x) -> p i y x", i=cL, y=10, x=10
                        )
                        nc.scalar.activation(
                            s6i[:, mc, i0 : i0 + cL, 1:9, 1:9],
                            pv[:, :, 0:8, 0:8],
                            Sign, bias=tbias(5, mc), scale=1.0,
                        )

            # ------------- L6: binconv 512->512 (DoubleRow), pool, BN6, sign
            for mc in range(4):
                for cb in range(0, nchk, 3):
                    bs = min(3, nchk - cb)
                    pss = [
                        psum.tile([128, cL * 100], F32, tag=f"ps{j}", name=f"ps{j}")
                        for j in range(bs)
                    ]
                    k = 0
                    for kp in range(2):
                        for t, (dy, dx) in enumerate(TAPS9):
                            for j in range(bs):
                                o = (cb + j) * cL * 100 + dy * 10 + dx
                                nc.tensor.matmul(
                                    pss[j], wl6[:, t, 2 * kp : 2 * kp + 2, mc, :],
                                    s6f[:, 2 * kp : 2 * kp + 2, o : o + cL * 100],
                                    start=(k == 0), stop=(k == 17), perf_mode=DR,
                                )
                            k += 1
                    for j in range(bs):
                        i0 = (cb + j) * cL
                        pv = pss[j].rearrange(
                            "p (i y x) -> p i y x", i=cL, y=10, x=10
                        )
                        t1 = post.tile([128, cL, 8, 4], F32, tag="t1")
                        nc.vector.reduce_max(
                            t1,
                            pv[:, :, 0:8, 0:8].rearrange(
                                "p i y (x two) -> p i y x two", two=2
                            ),
                            axis=AX,
                        )
                        t2 = post.tile([128, cL, 4, 4], F32, tag="t2")
                        nc.vector.reduce_max(
                            t2, t1.rearrange("p i (y two) x -> p i y x two", two=2),
                            axis=AX,
                        )
                        nc.scalar.activation(
                            s7[:, mc, i0 : i0 + cL, :, :], t2, Sign,
                            bias=tbias(6, mc), scale=1.0,
                        )

            # ------------- L7: binconv 512->10 (4x4), relu, BN7, softmax
            s7v = s7.rearrange("p k i y x -> p k i (y x)")
            p7 = psum7.tile([10, g], F32, tag="p7")
            k = 0
            for t in range(16):
                for kc in range(4):
                    o = _OFF[7] + (t * 4 + kc) * 10
                    nc.tensor.matmul(
                        p7, wsb[:, o : o + 10], s7v[:, kc, :, t],
                        start=(k == 0), stop=(k == 63),
                    )
                    k += 1
            h7 = post.tile([10, g], F32, tag="h7")
            nc.vector.tensor_scalar_max(h7, p7, 0.0)
            v7 = post.tile([10, g], F32, tag="v7")
            nc.scalar.activation(
                v7, h7, Identity, bias=bn7sb[:, 1:2], scale=bn7sb[:, 0:1]
            )
            pt = psum7.tile([g, 10], F32, tag="pt")
            nc.tensor.transpose(pt, v7, ident)
            mx = post.tile([g, 1], F32, tag="mx")
            nc.vector.reduce_max(mx, pt, axis=AX)
            nmx = post.tile([g, 1], F32, tag="nmx")
            nc.vector.tensor_scalar_mul(nmx, mx, -1.0)
            ex = post.tile([g, 10], F32, tag="ex")
            nc.scalar.activation(ex, pt, Exp, bias=nmx, scale=1.0)
            sm = post.tile([g, 1], F32, tag="sm")
            nc.vector.reduce_sum(sm, ex, axis=AX)
            ri = post.tile([g, 1], F32, tag="ri")
            nc.vector.reciprocal(ri, sm)
            yo = post.tile([g, 10], F32, tag="yo")
            nc.vector.tensor_scalar_mul(yo, ex, ri)
            nc.sync.dma_start(out=y[i00 : i00 + g, :], in_=yo)

    nc.compile()
    return nc


# ------------------------------------------------------------------ host prep

def _thresh_bias(gm, be, m, v):
    """bias such that next-layer input = Sign(pre_bn_value + bias)."""
    a = gm.astype(np.float64) / np.sqrt(v.astype(np.float64) + EPS)
    c = be.astype(np.float64) - a * m.astype(np.float64)
    return np.where(c < 0.0, c / a, BIG).astype(np.float32)  # -T = c/a


def _pack_w(wl):
    """sign(w) [3,3,Cin,Cout] -> [128, 9*KC*MC*128] fp8, (tap,kc,mc,q) order."""
    s = np.where(wl >= 0, 1.0, -1.0).astype(np.float32)
    _, _, cin, cout = wl.shape
    kc, mcn = cin // 128, cout // 128
    a = s.reshape(3, 3, kc, 128, mcn, 128)
    a = np.ascontiguousarray(a.transpose(3, 0, 1, 2, 4, 5))
    return a.reshape(128, 9 * kc * mcn * 128).astype(NP8)


def _prep_shared(inputs):
    d = {k: np.asarray(v, np.float32) for k, v in inputs.items()}

    wall = np.empty((128, WTOT), dtype=NP8)
    for layer in (2, 3, 4, 5, 6):
        wl = _pack_w(d[f"w{layer}"])
        wall[:, _OFF[layer] : _OFF[layer] + wl.shape[1]] = wl
    s7w = np.where(d["w7"] >= 0, 1.0, -1.0).astype(np.float32)
    a = s7w.reshape(4, 4, 4, 128, 10).transpose(3, 0, 1, 2, 4)
    wall[:, _OFF[7] :] = np.ascontiguousarray(a).reshape(128, 640).astype(NP8)

    cvec = np.zeros((128, 14), dtype=np.float32)
    tb1 = _thresh_bias(d["g1"], d["be1"], d["m1"], d["v1"])
    cvec[:, 0] = (d["b1"].astype(np.float64) + tb1.astype(np.float64)).astype(
        np.float32
    )
    for layer in (2, 3, 4, 5, 6):
        tb = _thresh_bias(
            d[f"g{layer}"], d[f"be{layer}"], d[f"m{layer}"], d[f"v{layer}"]
        )
        cvec[:, CVCOL[layer] : CVCOL[layer] + MC[layer]] = tb.reshape(
            MC[layer], 128
        ).T

    a7 = d["g7"].astype(np.float64) / np.sqrt(d["v7"].astype(np.float64) + EPS)
    c7 = d["be7"].astype(np.float64) - a7 * d["m7"].astype(np.float64)
    bn7 = np.stack([a7.astype(np.float32), c7.astype(np.float32)], axis=1)

    w1r = np.ascontiguousarray(d["w1"].reshape(27, 128))
    return d, wall, cvec, bn7, w1r


def _im2col(x):
    """x [B,32,32,3] -> [27, B, 900] f32, row order (dy,dx,c)."""
    from numpy.lib.stride_tricks import sliding_window_view

    sw = sliding_window_view(x, (3, 3), axis=(1, 2))  # [B,30,30,3,3,3]
    im = sw.transpose(4, 5, 3, 0, 1, 2).reshape(27, x.shape[0], 900)
    return np.ascontiguousarray(im)


LAST_RESULTS = None


def kernel(**inputs):
    global LAST_RESULTS
    nb, g = NB, 32
    key = (nb, g)
    if key not in _prog_cache:
        _prog_cache[key] = build_program(nb, g)
    nc = _prog_cache[key]

    d, wall, cvec, bn7, w1r = _prep_shared(inputs)
    im = _im2col(d["x"])

    in_maps = []
    for c in range(NCORES):
        xi = np.ascontiguousarray(im[:, c * nb : (c + 1) * nb, :]).reshape(
            27, nb * 900
        )
        in_maps.append(
            {"x1": xi, "w1": w1r, "wall": wall, "cvec": cvec, "bn7": bn7}
        )

    trace = bool(int(os.environ.get("KERNEL_TRACE", "0")))
    res = run_bass_kernel_spmd(
        nc, in_maps, core_ids=list(range(NCORES)), trace=trace
    )
    LAST_RESULTS = res
    out = np.concatenate([res.results[i]["y"] for i in range(NCORES)], axis=0)
    return out.astype(np.float32)


# revision 12
# speedup vs baseline: 1.3000x; 1.1027x over previous
"""Trainium2 Bass kernel for a 7-layer Riptide-style binarized CNN.

Strategy (data-parallel over 8 NeuronCores, 64 images/core):
  - conv1 (full precision) is one K=27 float32r matmul per 450 output
    positions, from a host-built im2col matrix [27, 64*900].
  - Every BN(+relu)(+maxpool)->sign boundary folds into a per-output-
    channel threshold: next layer's +-1 input is Sign(psum + bias) on the
    scalar engine straight out of PSUM (bias = -T, or +BIG for always-+1
    channels).  maxpool commutes with relu and monotone BN, so pooling
    runs on raw PSUM integers (two strided reduce_max ops) before Sign.
  - conv2..7 operands are +-1 fp8e4m3; PSUM accumulates exact integer
    counts in fp32, so the binary convs are exact.
  - Zero-padding happens before sign, so pad regions are sign(0)=+1: the
    padded activation buffers are memset to +1 once; only interiors are
    rewritten.
  - L4/L5/L6 (Cin>=256) use fp8 DoubleRow: contraction-pairs (c, c+128)
    live in one partition as [128, 2, span] flat layouts; the conv runs
    on the full padded grid (garbage at right/bottom edges discarded by
    the strided post-op reads), so the moving AP is a single contiguous
    span and each matmul does 2 taps-worth of MACs per cycle.
  - Images stream in groups of 32; weights are resident in SBUF.
"""

import os
import sys

sys.path.insert(0, "/opt/trn_rl_repo")

import numpy as np
import ml_dtypes
from contextlib import ExitStack

import concourse.bass as bass  # noqa: F401
import concourse.mybir as mybir
import concourse.tile as tile
from concourse import bacc
from concourse.bass_utils import run_bass_kernel_spmd
from concourse.masks import make_identity

F32 = mybir.dt.float32
F32R = mybir.dt.float32r
FP8 = mybir.dt.float8e4
NP8 = ml_dtypes.float8_e4m3fn
DR = mybir.MatmulPerfMode.DoubleRow

NCORES = 8
B = 512
NB = B // NCORES
EPS = 1e-3
BIG = 1e30

TAPS9 = [(dy, dx) for dy in range(3) for dx in range(3)]

KC = {2: 1, 3: 1, 4: 2, 5: 2, 6: 4}
MC = {2: 1, 3: 2, 4: 2, 5: 4, 6: 4}

_OFF = {}
_o = 0
for _l in (2, 3, 4, 5, 6):
    _OFF[_l] = _o
    _o += 9 * KC[_l] * MC[_l] * 128
_OFF[7] = _o
WTOT = _o + 16 * 4 * 10

CVCOL = {1: 0, 2: 1, 3: 2, 4: 4, 5: 6, 6: 10}

_prog_cache = {}


def _woff(layer, tap, kc, mc):
    return _OFF[layer] + ((tap * KC[layer] + kc) * MC[layer] + mc) * 128


def build_program(nb=NB, g=32):
    assert nb % g == 0
    c4 = min(4, g)   # L1 dma chunk, images
    c2 = min(2, g)   # L3 chunk, images
    cL = min(4, g)   # L5/L6 chunk, images (full-grid span 4*100=400)
    assert g % c4 == 0 and g % cL == 0

    span4 = g * 324 + 48   # flat padded span per kc block (16B aligned)
    span5 = g * 100 + 32
    span6 = g * 100 + 32

    nc = bacc.Bacc("TRN2", target_bir_lowering=False, debug=False)
    Sign = mybir.ActivationFunctionType.Sign
    Exp = mybir.ActivationFunctionType.Exp
    Identity = mybir.ActivationFunctionType.Identity
    AX = mybir.AxisListType.X

    x1 = nc.declare_dram_parameter("x1", [27, nb * 900], F32, isOutput=False)
    w1 = nc.declare_dram_parameter("w1", [128, 128], F32, isOutput=False)
    wall = nc.declare_dram_parameter("wall", [128, WTOT], FP8, isOutput=False)
    cvec = nc.declare_dram_parameter("cvec", [128, 14], F32, isOutput=False)
    bn7 = nc.declare_dram_parameter("bn7", [10, 2], F32, isOutput=False)
    y = nc.declare_dram_parameter("y", [nb, 10], F32, isOutput=True)

    with tile.TileContext(nc) as tc, ExitStack() as ctx:
        consts = ctx.enter_context(tc.tile_pool(name="consts", bufs=1))
        sbufs = ctx.enter_context(tc.tile_pool(name="sbufs", bufs=1))
        xpool = ctx.enter_context(tc.tile_pool(name="xpool", bufs=3))
        post = ctx.enter_context(tc.tile_pool(name="post", bufs=4))
        psum = ctx.enter_context(tc.tile_pool(name="psum", bufs=2, space="PSUM"))
        psum7 = ctx.enter_context(tc.tile_pool(name="psum7", bufs=1, space="PSUM"))

        w1sb = consts.tile([128, 128], F32)
        nc.sync.dma_start(out=w1sb, in_=w1[:, :])
        wsb = consts.tile([128, WTOT], FP8)
        nc.sync.dma_start(out=wsb, in_=wall[:, :])
        cv = consts.tile([128, 14], F32)
        nc.sync.dma_start(out=cv, in_=cvec[:, :])
        bn7sb = consts.tile([10, 2], F32)
        nc.sync.dma_start(out=bn7sb, in_=bn7[:, :])
        ident = consts.tile([10, 10], F32)
        make_identity(nc, ident)

        # DoubleRow weight views: [128, (tap), (kc), (mc), 128]
        def wview(layer):
            n = 9 * KC[layer] * MC[layer] * 128
            return wsb[:, _OFF[layer] : _OFF[layer] + n].rearrange(
                "p (t k m q) -> p t k m q",
                t=9, k=KC[layer], m=MC[layer], q=128,
            )

        wl4, wl5, wl6 = wview(4), wview(5), wview(6)

        # persistent activation buffers (one group's worth, reused)
        s2 = sbufs.tile([128, g, 34, 34], FP8)
        s3 = sbufs.tile([128, g, 18, 18], FP8)
        s4f = sbufs.tile([128, 2, span4], FP8)
        s5f = sbufs.tile([128, 2, span5], FP8)
        s6f = sbufs.tile([128, 4, span6], FP8)
        s7 = sbufs.tile([128, 4, g, 4, 4], FP8)
        s4i = s4f[:, :, : g * 324].rearrange(
            "p k (i y x) -> p k i y x", i=g, y=18, x=18
        )
        s5i = s5f[:, :, : g * 100].rearrange(
            "p k (i y x) -> p k i y x", i=g, y=10, x=10
        )
        s6i = s6f[:, :, : g * 100].rearrange(
            "p k (i y x) -> p k i y x", i=g, y=10, x=10
        )
        for t in (s2, s3, s4f, s5f, s6f):
            nc.gpsimd.memset(t, 1.0)

        def w8(layer, tap, kc, mc):
            o = _woff(layer, tap, kc, mc)
            return wsb[:, o : o + 128]

        def tbias(layer, mc):
            c = CVCOL[layer] + mc
            return cv[:, c : c + 1]

        for grp in range(nb // g):
            i00 = grp * g

            # ------------- L1: conv1 (fp32, 4x row-tiled) + bias/relu/BN1/sign
            # 4 concurrent 32-row PE strips, each on its own 450-pos chunk.
            for ch in range(g // c4):
                xt = xpool.tile([128, 2, 450], F32, tag="xt")
                base = (i00 + ch * c4) * 900
                for st in range(4):
                    nc.sync.dma_start(
                        out=xt[32 * st : 32 * st + 27, :, :].rearrange(
                            "p a b -> p (a b)"
                        ),
                        in_=x1[:, base + st * 900 : base + (st + 1) * 900],
                    )
                for sc in range(2 * c4):
                    st, half = sc // 2, sc % 2
                    p = psum.tile([128, 15, 30], F32, tag=f"ps{sc % 3}", name=f"ps{sc % 3}")
                    nc.tensor.matmul(
                        p, w1sb[32 * st : 32 * st + 27, :],
                        xt[32 * st : 32 * st + 27, half, :],
                        start=True, stop=True, tile_position=(32 * st, 0),
                    )
                    img = ch * c4 + st
                    r0 = half * 15
                    nc.scalar.activation(
                        s2[:, img, 2 + r0 : 17 + r0, 2:32], p, Sign,
                        bias=tbias(1, 0), scale=1.0,
                    )

            # ------------- L2: binconv 128->128, pool, BN2, pad, sign
            for img in range(g):
                for rc in range(2):
                    p = psum.tile([128, 16, 32], F32, tag=f"ps{(2 * img + rc) % 3}")
                    for t, (dy, dx) in enumerate(TAPS9):
                        nc.tensor.matmul(
                            p, w8(2, t, 0, 0),
                            s2[:, img, rc * 16 + dy : rc * 16 + dy + 16, dx : dx + 32],
                            start=(t == 0), stop=(t == 8),
                        )
                    t1 = post.tile([128, 16, 16], F32, tag="t1")
                    nc.vector.reduce_max(
                        t1, p.rearrange("p y (x two) -> p y x two", two=2), axis=AX
                    )
                    t2 = post.tile([128, 8, 16], F32, tag="t2")
                    nc.vector.reduce_max(
                        t2, t1.rearrange("p (y two) x -> p y x two", two=2), axis=AX
                    )
                    nc.scalar.activation(
                        s3[:, img, 1 + rc * 8 : 9 + rc * 8, 1:17], t2, Sign,
                        bias=tbias(2, 0), scale=1.0,
                    )

            # ------------- L3: binconv 128->256, BN3, pad, sign
            for chk in range(g // c2):
                i0 = chk * c2
                for mc in range(2):
                    p = psum.tile([128, c2, 16, 16], F32, tag=f"ps{(2 * chk + mc) % 3}")
                    for t, (dy, dx) in enumerate(TAPS9):
                        nc.tensor.matmul(
                            p, w8(3, t, 0, mc),
                            s3[:, i0 : i0 + c2, dy : dy + 16, dx : dx + 16],
                            start=(t == 0), stop=(t == 8),
                        )
                    nc.scalar.activation(
                        s4i[:, mc, i0 : i0 + c2, 1:17, 1:17], p, Sign,
                        bias=tbias(3, mc), scale=1.0,
                    )

            # ------------- L4: binconv 256->256 (DoubleRow), pool, BN4, sign
            # weight-reuse: each (tap, mc) weight load feeds 3 psum accumulators
            for mc in range(2):
                for b0 in range(0, g, 3):
                    bs = min(3, g - b0)
                    pss = [
                        psum.tile([128, 324], F32, tag=f"ps{j}", name=f"ps{j}")
                        for j in range(bs)
                    ]
                    for t, (dy, dx) in enumerate(TAPS9):
                        for j in range(bs):
                            o = (b0 + j) * 324 + dy * 18 + dx
                            nc.tensor.matmul(
                                pss[j], wl4[:, t, 0:2, mc, :],
                                s4f[:, :, o : o + 324],
                                start=(t == 0), stop=(t == 8), perf_mode=DR,
                            )
                    for j in range(bs):
                        pv = pss[j].rearrange("p (y x) -> p y x", y=18, x=18)
                        t1 = post.tile([128, 16, 8], F32, tag="t1")
                        nc.vector.reduce_max(
                            t1,
                            pv[:, 0:16, 0:16].rearrange(
                                "p y (x two) -> p y x two", two=2
                            ),
                            axis=AX,
                        )
                        t2 = post.tile([128, 8, 8], F32, tag="t2")
                        nc.vector.reduce_max(
                            t2, t1.rearrange("p (y two) x -> p y x two", two=2),
                            axis=AX,
                        )
                        nc.scalar.activation(
                            s5i[:, mc, b0 + j, 1:9, 1:9], t2, Sign,
                            bias=tbias(4, mc), scale=1.0,
                        )

            # ------------- L5: binconv 256->512 (DoubleRow), BN5, pad, sign
            nchk = g // cL
            for mc in range(4):
                for cb in range(0, nchk, 3):
                    bs = min(3, nchk - cb)
                    pss = [
                        psum.tile([128, cL * 100], F32, tag=f"ps{j}", name=f"ps{j}")
                        for j in range(bs)
                    ]
                    for t, (dy, dx) in enumerate(TAPS9):
                        for j in range(bs):
                            o = (cb + j) * cL * 100 + dy * 10 + dx
                            nc.tensor.matmul(
                                pss[j], wl5[:, t, 0:2, mc, :],
                                s5f[:, :, o : o + cL * 100],
                                start=(t == 0), stop=(t == 8), perf_mode=DR,
                            )
                    for j in range(bs):
                        i0 = (cb + j) * cL
                        pv = pss[j].rearrange(
                            "p (i y x) -> p i y x", i=cL, y=10, x=10
                        )
                        nc.scalar.activation(
                            s6i[:, mc, i0 : i0 + cL, 1:9, 1:9],
                            pv[:, :, 0:8, 0:8],
                            Sign, bias=tbias(5, mc), scale=1.0,
                        )

            # ------------- L6: binconv 512->512 (DoubleRow), pool, BN6, sign
            for mc in range(4):
                for cb in range(0, nchk, 3):
                    bs = min(3, nchk - cb)
                    pss = [
                        psum.tile([128, cL * 100], F32, tag=f"ps{j}", name=f"ps{j}")
                        for j in range(bs)
                    ]
                    k = 0
                    for kp in range(2):
                        for t, (dy, dx) in enumerate(TAPS9):
                            for j in range(bs):
                                o = (cb + j) * cL * 100 + dy * 10 + dx
                                nc.tensor.matmul(
                                    pss[j], wl6[:, t, 2 * kp : 2 * kp + 2, mc, :],
                                    s6f[:, 2 * kp : 2 * kp + 2, o : o + cL * 100],
                                    start=(k == 0), stop=(k == 17), perf_mode=DR,
                                )
                            k += 1
                    for j in range(bs):
                        i0 = (cb + j) * cL
                        pv = pss[j].rearrange(
                            "p (i y x) -> p i y x", i=cL, y=10, x=10
                        )
                        t1 = post.tile([128, cL, 8, 4], F32, tag="t1")
                        nc.vector.reduce_max(
                            t1,
                            pv[:, :, 0:8, 0:8].rearrange(
                                "p i y (x two) -> p i y x two", two=2
                            ),
                            axis=AX,
                        )
                        t2 = post.tile([128, cL, 4, 4], F32, tag="t2")
                        nc.vector.reduce_max(
                            t2, t1.rearrange("p i (y two) x -> p i y x two", two=2),
                            axis=AX,
                        )
                        nc.scalar.activation(
                            s7[:, mc, i0 : i0 + cL, :, :], t2, Sign,
                            bias=tbias(6, mc), scale=1.0,
                        )

            # ------------- L7: binconv 512->10 (4x4), relu, BN7, softmax
            s7v = s7.rearrange("p k i y x -> p k i (y x)")
            p7 = psum7.tile([10, g], F32, tag="p7")
            k = 0
            for t in range(16):
                for kc in range(4):
                    o = _OFF[7] + (t * 4 + kc) * 10
                    nc.tensor.matmul(
                        p7, wsb[:, o : o + 10], s7v[:, kc, :, t],
                        start=(k == 0), stop=(k == 63),
                    )
                    k += 1
            h7 = post.tile([10, g], F32, tag="h7")
            nc.vector.tensor_scalar_max(h7, p7, 0.0)
            v7 = post.tile([10, g], F32, tag="v7")
            nc.scalar.activation(
                v7, h7, Identity, bias=bn7sb[:, 1:2], scale=bn7sb[:, 0:1]
            )
            pt = psum7.tile([g, 10], F32, tag="pt")
            nc.tensor.transpose(pt, v7, ident)
            mx = post.tile([g, 1], F32, tag="mx")
            nc.vector.reduce_max(mx, pt, axis=AX)
            nmx = post.tile([g, 1], F32, tag="nmx")
            nc.vector.tensor_scalar_mul(nmx, mx, -1.0)
            ex = post.tile([g, 10], F32, tag="ex")
            nc.scalar.activation(ex, pt, Exp, bias=nmx, scale=1.0)
            sm = post.tile([g, 1], F32, tag="sm")
            nc.vector.reduce_sum(sm, ex, axis=AX)
            ri = post.tile([g, 1], F32, tag="ri")
            nc.vector.reciprocal(ri, sm)
            yo = post.tile([g, 10], F32, tag="yo")
            nc.vector.tensor_scalar_mul(yo, ex, ri)
            nc.sync.dma_start(out=y[i00 : i00 + g, :], in_=yo)

    nc.compile()
    return nc


# ------------------------------------------------------------------ host prep

def _thresh_bias(gm, be, m, v):
    """bias such that next-layer input = Sign(pre_bn_value + bias)."""
    a = gm.astype(np.float64) / np.sqrt(v.astype(np.float64) + EPS)
    c = be.astype(np.float64) - a * m.astype(np.float64)
    return np.where(c < 0.0, c / a, BIG).astype(np.float32)  # -T = c/a


def _pack_w(wl):
    """sign(w) [3,3,Cin,Cout] -> [128, 9*KC*MC*128] fp8, (tap,kc,mc,q) order."""
    s = np.where(wl >= 0, 1.0, -1.0).astype(np.float32)
    _, _, cin, cout = wl.shape
    kc, mcn = cin // 128, cout // 128
    a = s.reshape(3, 3, kc, 128, mcn, 128)
    a = np.ascontiguousarray(a.transpose(3, 0, 1, 2, 4, 5))
    return a.reshape(128, 9 * kc * mcn * 128).astype(NP8)


def _prep_shared(inputs):
    d = {k: np.asarray(v, np.float32) for k, v in inputs.items()}

    wall = np.empty((128, WTOT), dtype=NP8)
    for layer in (2, 3, 4, 5, 6):
        wl = _pack_w(d[f"w{layer}"])
        wall[:, _OFF[layer] : _OFF[layer] + wl.shape[1]] = wl
    s7w = np.where(d["w7"] >= 0, 1.0, -1.0).astype(np.float32)
    a = s7w.reshape(4, 4, 4, 128, 10).transpose(3, 0, 1, 2, 4)
    wall[:, _OFF[7] :] = np.ascontiguousarray(a).reshape(128, 640).astype(NP8)

    cvec = np.zeros((128, 14), dtype=np.float32)
    tb1 = _thresh_bias(d["g1"], d["be1"], d["m1"], d["v1"])
    cvec[:, 0] = (d["b1"].astype(np.float64) + tb1.astype(np.float64)).astype(
        np.float32
    )
    for layer in (2, 3, 4, 5, 6):
        tb = _thresh_bias(
            d[f"g{layer}"], d[f"be{layer}"], d[f"m{layer}"], d[f"v{layer}"]
        )
        cvec[:, CVCOL[layer] : CVCOL[layer] + MC[layer]] = tb.reshape(
            MC[layer], 128
        ).T

    a7 = d["g7"].astype(np.float64) / np.sqrt(d["v7"].astype(np.float64) + EPS)
    c7 = d["be7"].astype(np.float64) - a7 * d["m7"].astype(np.float64)
    bn7 = np.stack([a7.astype(np.float32), c7.astype(np.float32)], axis=1)

    w1r = np.zeros((128, 128), dtype=np.float32)
    for st in range(4):
        w1r[32 * st : 32 * st + 27, :] = d["w1"].reshape(27, 128)
    return d, wall, cvec, bn7, w1r


def _im2col(x):
    """x [B,32,32,3] -> [27, B, 900] f32, row order (dy,dx,c)."""
    from numpy.lib.stride_tricks import sliding_window_view

    sw = sliding_window_view(x, (3, 3), axis=(1, 2))  # [B,30,30,3,3,3]
    im = sw.transpose(4, 5, 3, 0, 1, 2).reshape(27, x.shape[0], 900)
    return np.ascontiguousarray(im)


LAST_RESULTS = None


def kernel(**inputs):
    global LAST_RESULTS
    nb, g = NB, 32
    key = (nb, g)
    if key not in _prog_cache:
        _prog_cache[key] = build_program(nb, g)
    nc = _prog_cache[key]

    d, wall, cvec, bn7, w1r = _prep_shared(inputs)
    im = _im2col(d["x"])

    in_maps = []
    for c in range(NCORES):
        xi = np.ascontiguousarray(im[:, c * nb : (c + 1) * nb, :]).reshape(
            27, nb * 900
        )
        in_maps.append(
            {"x1": xi, "w1": w1r, "wall": wall, "cvec": cvec, "bn7": bn7}
        )

    trace = bool(int(os.environ.get("KERNEL_TRACE", "0")))
    res = run_bass_kernel_spmd(
        nc, in_maps, core_ids=list(range(NCORES)), trace=trace
    )
    LAST_RESULTS = res
    out = np.concatenate([res.results[i]["y"] for i in range(NCORES)], axis=0)
    return out.astype(np.float32)


# revision 14
# speedup vs baseline: 1.4535x; 1.1181x over previous
"""Trainium2 Bass kernel for a 7-layer Riptide-style binarized CNN.

Strategy (data-parallel over 8 NeuronCores, 64 images/core):
  - conv1 (full precision) is one K=27 float32r matmul per 450 output
    positions, from a host-built im2col matrix [27, 64*900].
  - Every BN(+relu)(+maxpool)->sign boundary folds into a per-output-
    channel threshold: next layer's +-1 input is Sign(psum + bias) on the
    scalar engine straight out of PSUM (bias = -T, or +BIG for always-+1
    channels).  maxpool commutes with relu and monotone BN, so pooling
    runs on raw PSUM integers (two strided reduce_max ops) before Sign.
  - conv2..7 operands are +-1 fp8e4m3; PSUM accumulates exact integer
    counts in fp32, so the binary convs are exact.
  - Zero-padding happens before sign, so pad regions are sign(0)=+1: the
    padded activation buffers are memset to +1 once; only interiors are
    rewritten.
  - L4/L5/L6 (Cin>=256) use fp8 DoubleRow: contraction-pairs (c, c+128)
    live in one partition as [128, 2, span] flat layouts; the conv runs
    on the full padded grid (garbage at right/bottom edges discarded by
    the strided post-op reads), so the moving AP is a single contiguous
    span and each matmul does 2 taps-worth of MACs per cycle.
  - Images stream in groups of 32; weights are resident in SBUF.
"""

import os
import sys

sys.path.insert(0, "/opt/trn_rl_repo")

import numpy as np
import ml_dtypes
from contextlib import ExitStack

import concourse.bass as bass  # noqa: F401
import concourse.mybir as mybir
import concourse.tile as tile
from concourse import bacc
from concourse.bass_utils import run_bass_kernel_spmd
from concourse.masks import make_identity

F32 = mybir.dt.float32
F32R = mybir.dt.float32r
FP8 = mybir.dt.float8e4
NP8 = ml_dtypes.float8_e4m3fn
DR = mybir.MatmulPerfMode.DoubleRow

NCORES = 8
B = 512
NB = B // NCORES
EPS = 1e-3
BIG = 1e30

TAPS9 = [(dy, dx) for dy in range(3) for dx in range(3)]

KC = {2: 1, 3: 1, 4: 2, 5: 2, 6: 4}
MC = {2: 1, 3: 2, 4: 2, 5: 4, 6: 4}

_OFF = {}
_o = 0
for _l in (2, 3, 4, 5, 6):
    _OFF[_l] = _o
    _o += 9 * KC[_l] * MC[_l] * 128
_OFF[7] = _o
WTOT = _o + 16 * 4 * 10

CVCOL = {1: 0, 2: 1, 3: 2, 4: 4, 5: 6, 6: 10}

_prog_cache = {}


def _woff(layer, tap, kc, mc):
    return _OFF[layer] + ((tap * KC[layer] + kc) * MC[layer] + mc) * 128


def build_program(nb=NB, g=16):
    assert nb % g == 0
    c4 = min(4, g)   # L1 dma chunk, images
    c2 = min(2, g)   # L3 chunk, images
    cL = min(4, g)   # L5/L6 chunk, images (full-grid span 4*100=400)
    assert g % c4 == 0 and g % cL == 0

    span4 = g * 324 + 48   # flat padded span per kc block (16B aligned)
    span5 = g * 100 + 32
    span6 = g * 100 + 32

    nc = bacc.Bacc("TRN2", target_bir_lowering=False, debug=False)
    Sign = mybir.ActivationFunctionType.Sign
    Exp = mybir.ActivationFunctionType.Exp
    Identity = mybir.ActivationFunctionType.Identity
    AX = mybir.AxisListType.X

    x1 = nc.declare_dram_parameter("x1", [27, nb * 900], F32, isOutput=False)
    w1 = nc.declare_dram_parameter("w1", [128, 128], F32, isOutput=False)
    wall = nc.declare_dram_parameter("wall", [128, WTOT], FP8, isOutput=False)
    cvec = nc.declare_dram_parameter("cvec", [128, 14], F32, isOutput=False)
    wallp = nc.declare_dram_parameter("wallp", [128, 3456], FP8, isOutput=False)
    bn7 = nc.declare_dram_parameter("bn7", [10, 2], F32, isOutput=False)
    y = nc.declare_dram_parameter("y", [nb, 10], F32, isOutput=True)

    with tile.TileContext(nc) as tc, ExitStack() as ctx:
        consts = ctx.enter_context(tc.tile_pool(name="consts", bufs=1))
        sbufs = ctx.enter_context(tc.tile_pool(name="sbufs", bufs=1))
        xpool = ctx.enter_context(tc.tile_pool(name="xpool", bufs=3))
        post = ctx.enter_context(tc.tile_pool(name="post", bufs=4))
        psum = ctx.enter_context(tc.tile_pool(name="psum", bufs=2, space="PSUM"))
        psum7 = ctx.enter_context(tc.tile_pool(name="psum7", bufs=1, space="PSUM"))

        w1sb = consts.tile([128, 128], F32)
        nc.sync.dma_start(out=w1sb, in_=w1[:, :])
        wsb = consts.tile([128, WTOT], FP8)
        nc.sync.dma_start(out=wsb, in_=wall[:, :])
        cv = consts.tile([128, 14], F32)
        nc.sync.dma_start(out=cv, in_=cvec[:, :])
        wpsb = consts.tile([128, 3456], FP8)
        nc.sync.dma_start(out=wpsb, in_=wallp[:, :])
        bn7sb = consts.tile([10, 2], F32)
        nc.sync.dma_start(out=bn7sb, in_=bn7[:, :])
        ident = consts.tile([10, 10], F32)
        make_identity(nc, ident)

        # DoubleRow weight views: [128, (tap), (kc), (mc), 128]
        def wview(layer):
            n = 9 * KC[layer] * MC[layer] * 128
            return wsb[:, _OFF[layer] : _OFF[layer] + n].rearrange(
                "p (t k m q) -> p t k m q",
                t=9, k=KC[layer], m=MC[layer], q=128,
            )

        wl4, wl5, wl6 = wview(4), wview(5), wview(6)
        # tap-pair weights: L2 pairs [3,2,128] @0, L2 singles [3,128] @768,
        # L3 pairs [3,2,2,128] @1152, L3 singles [3,2,128] @2688
        w2p = wpsb[:, 0:768].rearrange("p (d j q) -> p d j q", d=3, j=2, q=128)
        w2s = wpsb[:, 768:1152].rearrange("p (d q) -> p d q", d=3, q=128)
        w3p = wpsb[:, 1152:2688].rearrange(
            "p (d j m q) -> p d j m q", d=3, j=2, m=2, q=128
        )
        w3s = wpsb[:, 2688:3456].rearrange(
            "p (d m q) -> p d m q", d=3, m=2, q=128
        )

        # persistent activation buffers (one group's worth, reused)
        # s2d/s3d: copy j=0 is the padded sign grid; copy j=1 is the same
        # data shifted left by one column (B[o] = A[o+1]) so a DoubleRow
        # matmul pairs taps (dy,0)+(dy,1) with a single 16B-aligned stride.
        span2 = g * 1156 + 96
        span3 = g * 324 + 48
        s2d = sbufs.tile([128, 2, span2], FP8)
        s3d = sbufs.tile([128, 2, span3], FP8)
        s2i = s2d[:, :, : g * 1156].rearrange(
            "p k (i y x) -> p k i y x", i=g, y=34, x=34
        )
        s3i = s3d[:, :, : g * 324].rearrange(
            "p k (i y x) -> p k i y x", i=g, y=18, x=18
        )
        s4f = sbufs.tile([128, 2, span4], FP8)
        s5f = sbufs.tile([128, 2, span5], FP8)
        s6f = sbufs.tile([128, 4, span6], FP8)
        s7 = sbufs.tile([128, 4, g, 4, 4], FP8)
        s4i = s4f[:, :, : g * 324].rearrange(
            "p k (i y x) -> p k i y x", i=g, y=18, x=18
        )
        s5i = s5f[:, :, : g * 100].rearrange(
            "p k (i y x) -> p k i y x", i=g, y=10, x=10
        )
        s6i = s6f[:, :, : g * 100].rearrange(
            "p k (i y x) -> p k i y x", i=g, y=10, x=10
        )
        for t in (s2d, s3d, s4f, s5f, s6f):
            nc.gpsimd.memset(t, 1.0)

        def w8(layer, tap, kc, mc):
            o = _woff(layer, tap, kc, mc)
            return wsb[:, o : o + 128]

        def tbias(layer, mc):
            c = CVCOL[layer] + mc
            return cv[:, c : c + 1]

        for grp in range(nb // g):
            i00 = grp * g

            # ------------- L1: conv1 (fp32, 4x row-tiled) + bias/relu/BN1/sign
            # 4 concurrent 32-row PE strips, each on its own 450-pos chunk.
            for ch in range(g // c4):
                xt = xpool.tile([128, 2, 450], F32, tag="xt")
                base = (i00 + ch * c4) * 900
                for st in range(4):
                    nc.sync.dma_start(
                        out=xt[32 * st : 32 * st + 27, :, :].rearrange(
                            "p a b -> p (a b)"
                        ),
                        in_=x1[:, base + st * 900 : base + (st + 1) * 900],
                    )
                for sc in range(2 * c4):
                    st, half = sc // 2, sc % 2
                    p = psum.tile([128, 15, 30], F32, tag=f"ps{sc % 3}", name=f"ps{sc % 3}")
                    nc.tensor.matmul(
                        p, w1sb[32 * st : 32 * st + 27, :],
                        xt[32 * st : 32 * st + 27, half, :],
                        start=True, stop=True, tile_position=(32 * st, 0),
                    )
                    img = ch * c4 + st
                    r0 = half * 15
                    nc.scalar.activation(
                        s2i[:, 0, img, 2 + r0 : 17 + r0, 2:32], p, Sign,
                        bias=tbias(1, 0), scale=1.0,
                    )
                    nc.scalar.activation(
                        s2i[:, 1, img, 2 + r0 : 17 + r0, 1:31], p, Sign,
                        bias=tbias(1, 0), scale=1.0,
                    )

            # ------------- L2: binconv 128->128 (tap-pair DR), pool, BN2, sign
            for img in range(g):
                for rc in range(2):
                    p = psum.tile([128, 16, 32], F32, tag=f"ps{(2 * img + rc) % 3}")
                    k = 0
                    for dy in range(3):
                        r = rc * 16 + dy
                        nc.tensor.matmul(
                            p, w2p[:, dy, :, :],
                            s2i[:, :, img, r : r + 16, 0:32],
                            start=(k == 0), stop=False, perf_mode=DR,
                        )
                        k += 1
                        nc.tensor.matmul(
                            p, w2s[:, dy, :],
                            s2i[:, 0, img, r : r + 16, 2:34],
                            start=False, stop=(dy == 2),
                        )
                        k += 1
                    t1 = post.tile([128, 16, 16], F32, tag="t1")
                    nc.vector.reduce_max(
                        t1, p.rearrange("p y (x two) -> p y x two", two=2), axis=AX
                    )
                    t2 = post.tile([128, 8, 16], F32, tag="t2")
                    nc.vector.reduce_max(
                        t2, t1.rearrange("p (y two) x -> p y x two", two=2), axis=AX
                    )
                    nc.scalar.activation(
                        s3i[:, 0, img, 1 + rc * 8 : 9 + rc * 8, 1:17], t2, Sign,
                        bias=tbias(2, 0), scale=1.0,
                    )
                    nc.scalar.activation(
                        s3i[:, 1, img, 1 + rc * 8 : 9 + rc * 8, 0:16], t2, Sign,
                        bias=tbias(2, 0), scale=1.0,
                    )

            # ------------- L3: binconv 128->256 (tap-pair DR), BN3, pad, sign
            for img in range(g):
                for mc in range(2):
                    p = psum.tile([128, 16, 16], F32, tag=f"ps{(2 * img + mc) % 3}")
                    for dy in range(3):
                        nc.tensor.matmul(
                            p, w3p[:, dy, :, mc, :],
                            s3i[:, :, img, dy : dy + 16, 0:16],
                            start=(dy == 0), stop=False, perf_mode=DR,
                        )
                        nc.tensor.matmul(
                            p, w3s[:, dy, mc, :],
                            s3i[:, 0, img, dy : dy + 16, 2:18],
                            start=False, stop=(dy == 2),
                        )
                    nc.scalar.activation(
                        s4i[:, mc, img, 1:17, 1:17], p, Sign,
                        bias=tbias(3, mc), scale=1.0,
                    )

            # ------------- L4: binconv 256->256 (DoubleRow), pool, BN4, sign
            # weight-reuse: each (tap, mc) weight load feeds 3 psum accumulators
            for mc in range(2):
                for b0 in range(0, g, 3):
                    bs = min(3, g - b0)
                    pss = [
                        psum.tile([128, 16, 16], F32, tag=f"ps{j}", name=f"ps{j}")
                        for j in range(bs)
                    ]
                    for t, (dy, dx) in enumerate(TAPS9):
                        for j in range(bs):
                            nc.tensor.matmul(
                                pss[j], wl4[:, t, 0:2, mc, :],
                                s4i[:, :, b0 + j, dy : dy + 16, dx : dx + 16],
                                start=(t == 0), stop=(t == 8), perf_mode=DR,
                            )
                    for j in range(bs):
                        pv = pss[j]
                        t1 = post.tile([128, 16, 8], F32, tag="t1")
                        nc.vector.reduce_max(
                            t1,
                            pv.rearrange("p y (x two) -> p y x two", two=2),
                            axis=AX,
                        )
                        t2 = post.tile([128, 8, 8], F32, tag="t2")
                        nc.vector.reduce_max(
                            t2, t1.rearrange("p (y two) x -> p y x two", two=2),
                            axis=AX,
                        )
                        nc.scalar.activation(
                            s5i[:, mc, b0 + j, 1:9, 1:9], t2, Sign,
                            bias=tbias(4, mc), scale=1.0,
                        )

            # ------------- L5: binconv 256->512 (DoubleRow), BN5, pad, sign
            nchk = g // cL
            for mc in range(4):
                for cb in range(0, nchk, 3):
                    bs = min(3, nchk - cb)
                    pss = [
                        psum.tile([128, cL * 100], F32, tag=f"ps{j}", name=f"ps{j}")
                        for j in range(bs)
                    ]
                    for t, (dy, dx) in enumerate(TAPS9):
                        for j in range(bs):
                            o = (cb + j) * cL * 100 + dy * 10 + dx
                            nc.tensor.matmul(
                                pss[j], wl5[:, t, 0:2, mc, :],
                                s5f[:, :, o : o + cL * 100],
                                start=(t == 0), stop=(t == 8), perf_mode=DR,
                            )
                    for j in range(bs):
                        i0 = (cb + j) * cL
                        pv = pss[j].rearrange(
                            "p (i y x) -> p i y x", i=cL, y=10, x=10
                        )
                        nc.scalar.activation(
                            s6i[:, mc, i0 : i0 + cL, 1:9, 1:9],
                            pv[:, :, 0:8, 0:8],
                            Sign, bias=tbias(5, mc), scale=1.0,
                        )

            # ------------- L6: binconv 512->512 (DoubleRow), pool, BN6, sign
            for mc in range(4):
                for cb in range(0, nchk, 3):
                    bs = min(3, nchk - cb)
                    pss = [
                        psum.tile([128, cL * 100], F32, tag=f"ps{j}", name=f"ps{j}")
                        for j in range(bs)
                    ]
                    k = 0
                    for kp in range(2):
                        for t, (dy, dx) in enumerate(TAPS9):
                            for j in range(bs):
                                o = (cb + j) * cL * 100 + dy * 10 + dx
                                nc.tensor.matmul(
                                    pss[j], wl6[:, t, 2 * kp : 2 * kp + 2, mc, :],
                                    s6f[:, 2 * kp : 2 * kp + 2, o : o + cL * 100],
                                    start=(k == 0), stop=(k == 17), perf_mode=DR,
                                )
                            k += 1
                    for j in range(bs):
                        i0 = (cb + j) * cL
                        pv = pss[j].rearrange(
                            "p (i y x) -> p i y x", i=cL, y=10, x=10
                        )
                        t1 = post.tile([128, cL, 8, 4], F32, tag="t1")
                        nc.vector.reduce_max(
                            t1,
                            pv[:, :, 0:8, 0:8].rearrange(
                                "p i y (x two) -> p i y x two", two=2
                            ),
                            axis=AX,
                        )
                        t2 = post.tile([128, cL, 4, 4], F32, tag="t2")
                        nc.vector.reduce_max(
                            t2, t1.rearrange("p i (y two) x -> p i y x two", two=2),
                            axis=AX,
                        )
                        nc.scalar.activation(
                            s7[:, mc, i0 : i0 + cL, :, :], t2, Sign,
                            bias=tbias(6, mc), scale=1.0,
                        )

            # ------------- L7: binconv 512->10 (4x4), relu, BN7, softmax
            s7v = s7.rearrange("p k i y x -> p k i (y x)")
            p7 = psum7.tile([10, g], F32, tag="p7")
            k = 0
            for t in range(16):
                for kc in range(4):
                    o = _OFF[7] + (t * 4 + kc) * 10
                    nc.tensor.matmul(
                        p7, wsb[:, o : o + 10], s7v[:, kc, :, t],
                        start=(k == 0), stop=(k == 63),
                    )
                    k += 1
            h7 = post.tile([10, g], F32, tag="h7")
            nc.vector.tensor_scalar_max(h7, p7, 0.0)
            v7 = post.tile([10, g], F32, tag="v7")
            nc.scalar.activation(
                v7, h7, Identity, bias=bn7sb[:, 1:2], scale=bn7sb[:, 0:1]
            )
            pt = psum7.tile([g, 10], F32, tag="pt")
            nc.tensor.transpose(pt, v7, ident)
            mx = post.tile([g, 1], F32, tag="mx")
            nc.vector.reduce_max(mx, pt, axis=AX)
            nmx = post.tile([g, 1], F32, tag="nmx")
            nc.vector.tensor_scalar_mul(nmx, mx, -1.0)
            ex = post.tile([g, 10], F32, tag="ex")
            nc.scalar.activation(ex, pt, Exp, bias=nmx, scale=1.0)
            sm = post.tile([g, 1], F32, tag="sm")
            nc.vector.reduce_sum(sm, ex, axis=AX)
            ri = post.tile([g, 1], F32, tag="ri")
            nc.vector.reciprocal(ri, sm)
            yo = post.tile([g, 10], F32, tag="yo")
            nc.vector.tensor_scalar_mul(yo, ex, ri)
            nc.sync.dma_start(out=y[i00 : i00 + g, :], in_=yo)

    nc.compile()
    return nc


# ------------------------------------------------------------------ host prep

def _thresh_bias(gm, be, m, v):
    """bias such that next-layer input = Sign(pre_bn_value + bias)."""
    a = gm.astype(np.float64) / np.sqrt(v.astype(np.float64) + EPS)
    c = be.astype(np.float64) - a * m.astype(np.float64)
    return np.where(c < 0.0, c / a, BIG).astype(np.float32)  # -T = c/a


def _pack_w(wl):
    """sign(w) [3,3,Cin,Cout] -> [128, 9*KC*MC*128] fp8, (tap,kc,mc,q) order."""
    s = np.where(wl >= 0, 1.0, -1.0).astype(np.float32)
    _, _, cin, cout = wl.shape
    kc, mcn = cin // 128, cout // 128
    a = s.reshape(3, 3, kc, 128, mcn, 128)
    a = np.ascontiguousarray(a.transpose(3, 0, 1, 2, 4, 5))
    return a.reshape(128, 9 * kc * mcn * 128).astype(NP8)


def _prep_shared(inputs):
    d = {k: np.asarray(v, np.float32) for k, v in inputs.items()}

    wall = np.empty((128, WTOT), dtype=NP8)
    for layer in (2, 3, 4, 5, 6):
        wl = _pack_w(d[f"w{layer}"])
        wall[:, _OFF[layer] : _OFF[layer] + wl.shape[1]] = wl
    s7w = np.where(d["w7"] >= 0, 1.0, -1.0).astype(np.float32)
    a = s7w.reshape(4, 4, 4, 128, 10).transpose(3, 0, 1, 2, 4)
    wall[:, _OFF[7] :] = np.ascontiguousarray(a).reshape(128, 640).astype(NP8)

    cvec = np.zeros((128, 14), dtype=np.float32)
    tb1 = _thresh_bias(d["g1"], d["be1"], d["m1"], d["v1"])
    cvec[:, 0] = (d["b1"].astype(np.float64) + tb1.astype(np.float64)).astype(
        np.float32
    )
    for layer in (2, 3, 4, 5, 6):
        tb = _thresh_bias(
            d[f"g{layer}"], d[f"be{layer}"], d[f"m{layer}"], d[f"v{layer}"]
        )
        cvec[:, CVCOL[layer] : CVCOL[layer] + MC[layer]] = tb.reshape(
            MC[layer], 128
        ).T

    a7 = d["g7"].astype(np.float64) / np.sqrt(d["v7"].astype(np.float64) + EPS)
    c7 = d["be7"].astype(np.float64) - a7 * d["m7"].astype(np.float64)
    bn7 = np.stack([a7.astype(np.float32), c7.astype(np.float32)], axis=1)

    wp = np.empty((128, 3456), dtype=NP8)
    s2w = np.where(d["w2"] >= 0, 1.0, -1.0).astype(np.float32)
    s3w = np.where(d["w3"] >= 0, 1.0, -1.0).astype(np.float32)
    for dy in range(3):
        for j in range(2):
            wp[:, (dy * 2 + j) * 128 : (dy * 2 + j + 1) * 128] = s2w[dy, j].astype(NP8)
        wp[:, 768 + dy * 128 : 768 + (dy + 1) * 128] = s2w[dy, 2].astype(NP8)
        for j in range(2):
            for m in range(2):
                o = 1152 + ((dy * 2 + j) * 2 + m) * 128
                wp[:, o : o + 128] = s3w[dy, j, :, m * 128 : (m + 1) * 128].astype(NP8)
        for m in range(2):
            o = 2688 + (dy * 2 + m) * 128
            wp[:, o : o + 128] = s3w[dy, 2, :, m * 128 : (m + 1) * 128].astype(NP8)

    w1r = np.zeros((128, 128), dtype=np.float32)
    for st in range(4):
        w1r[32 * st : 32 * st + 27, :] = d["w1"].reshape(27, 128)
    return d, wall, wp, cvec, bn7, w1r


def _im2col(x):
    """x [B,32,32,3] -> [27, B, 900] f32, row order (dy,dx,c)."""
    from numpy.lib.stride_tricks import sliding_window_view

    sw = sliding_window_view(x, (3, 3), axis=(1, 2))  # [B,30,30,3,3,3]
    im = sw.transpose(4, 5, 3, 0, 1, 2).reshape(27, x.shape[0], 900)
    return np.ascontiguousarray(im)


LAST_RESULTS = None


def kernel(**inputs):
    global LAST_RESULTS
    nb, g = NB, 16
    key = (nb, g)
    if key not in _prog_cache:
        _prog_cache[key] = build_program(nb, g)
    nc = _prog_cache[key]

    d, wall, wp, cvec, bn7, w1r = _prep_shared(inputs)
    im = _im2col(d["x"])

    in_maps = []
    for c in range(NCORES):
        xi = np.ascontiguousarray(im[:, c * nb : (c + 1) * nb, :]).reshape(
            27, nb * 900
        )
        in_maps.append(
            {"x1": xi, "w1": w1r, "wall": wall, "wallp": wp, "cvec": cvec,
             "bn7": bn7}
        )

    trace = bool(int(os.environ.get("KERNEL_TRACE", "0")))
    res = run_bass_kernel_spmd(
        nc, in_maps, core_ids=list(range(NCORES)), trace=trace
    )
    LAST_RESULTS = res
    out = np.concatenate([res.results[i]["y"] for i in range(NCORES)], axis=0)
    return out.astype(np.float32)


# revision 15
# speedup vs baseline: 1.4625x; 1.0062x over previous
"""Trainium2 Bass kernel for a 7-layer Riptide-style binarized CNN.

Strategy (data-parallel over 8 NeuronCores, 64 images/core):
  - conv1 (full precision) is one K=27 float32r matmul per 450 output
    positions, from a host-built im2col matrix [27, 64*900].
  - Every BN(+relu)(+maxpool)->sign boundary folds into a per-output-
    channel threshold: next layer's +-1 input is Sign(psum + bias) on the
    scalar engine straight out of PSUM (bias = -T, or +BIG for always-+1
    channels).  maxpool commutes with relu and monotone BN, so pooling
    runs on raw PSUM integers (two strided reduce_max ops) before Sign.
  - conv2..7 operands are +-1 fp8e4m3; PSUM accumulates exact integer
    counts in fp32, so the binary convs are exact.
  - Zero-padding happens before sign, so pad regions are sign(0)=+1: the
    padded activation buffers are memset to +1 once; only interiors are
    rewritten.
  - L4/L5/L6 (Cin>=256) use fp8 DoubleRow: contraction-pairs (c, c+128)
    live in one partition as [128, 2, span] flat layouts; the conv runs
    on the full padded grid (garbage at right/bottom edges discarded by
    the strided post-op reads), so the moving AP is a single contiguous
    span and each matmul does 2 taps-worth of MACs per cycle.
  - Images stream in groups of 32; weights are resident in SBUF.
"""

import os
import sys

sys.path.insert(0, "/opt/trn_rl_repo")

import numpy as np
import ml_dtypes
from contextlib import ExitStack

import concourse.bass as bass  # noqa: F401
import concourse.mybir as mybir
import concourse.tile as tile
from concourse import bacc
from concourse.bass_utils import run_bass_kernel_spmd
from concourse.masks import make_identity

F32 = mybir.dt.float32
F32R = mybir.dt.float32r
FP8 = mybir.dt.float8e4
NP8 = ml_dtypes.float8_e4m3fn
DR = mybir.MatmulPerfMode.DoubleRow

NCORES = 8
B = 512
NB = B // NCORES
EPS = 1e-3
BIG = 1e30

TAPS9 = [(dy, dx) for dy in range(3) for dx in range(3)]

KC = {2: 1, 3: 1, 4: 2, 5: 2, 6: 4}
MC = {2: 1, 3: 2, 4: 2, 5: 4, 6: 4}

_OFF = {}
_o = 0
for _l in (2, 3, 4, 5, 6):
    _OFF[_l] = _o
    _o += 9 * KC[_l] * MC[_l] * 128
_OFF[7] = _o
WTOT = _o + 16 * 4 * 10

CVCOL = {1: 0, 2: 1, 3: 2, 4: 4, 5: 6, 6: 10}

_prog_cache = {}


def _woff(layer, tap, kc, mc):
    return _OFF[layer] + ((tap * KC[layer] + kc) * MC[layer] + mc) * 128


def build_program(nb=NB, g=16):
    assert nb % g == 0
    c4 = min(4, g)   # L1 dma chunk, images
    c2 = min(2, g)   # L3 chunk, images
    cL = min(4, g)   # L5/L6 chunk, images (full-grid span 4*100=400)
    assert g % c4 == 0 and g % cL == 0

    span4 = g * 324 + 48   # flat padded span per kc block (16B aligned)
    span5 = g * 100 + 32
    span6 = g * 100 + 32

    nc = bacc.Bacc("TRN2", target_bir_lowering=False, debug=False)
    Sign = mybir.ActivationFunctionType.Sign
    Exp = mybir.ActivationFunctionType.Exp
    Identity = mybir.ActivationFunctionType.Identity
    AX = mybir.AxisListType.X

    x1 = nc.declare_dram_parameter("x1", [27, nb * 900], F32, isOutput=False)
    w1 = nc.declare_dram_parameter("w1", [128, 128], F32, isOutput=False)
    wall = nc.declare_dram_parameter("wall", [128, WTOT], FP8, isOutput=False)
    cvec = nc.declare_dram_parameter("cvec", [128, 14], F32, isOutput=False)
    wallp = nc.declare_dram_parameter("wallp", [128, 3456], FP8, isOutput=False)
    bn7 = nc.declare_dram_parameter("bn7", [10, 2], F32, isOutput=False)
    y = nc.declare_dram_parameter("y", [nb, 10], F32, isOutput=True)

    with tile.TileContext(nc) as tc, ExitStack() as ctx:
        consts = ctx.enter_context(tc.tile_pool(name="consts", bufs=1))
        sbufs = ctx.enter_context(tc.tile_pool(name="sbufs", bufs=1))
        xpool = ctx.enter_context(tc.tile_pool(name="xpool", bufs=3))
        post = ctx.enter_context(tc.tile_pool(name="post", bufs=4))
        psum = ctx.enter_context(tc.tile_pool(name="psum", bufs=2, space="PSUM"))
        psum7 = ctx.enter_context(tc.tile_pool(name="psum7", bufs=1, space="PSUM"))

        w1sb = consts.tile([128, 128], F32)
        nc.sync.dma_start(out=w1sb, in_=w1[:, :])
        wsb = consts.tile([128, WTOT], FP8)
        nc.sync.dma_start(out=wsb, in_=wall[:, :])
        cv = consts.tile([128, 14], F32)
        nc.sync.dma_start(out=cv, in_=cvec[:, :])
        wpsb = consts.tile([128, 3456], FP8)
        nc.sync.dma_start(out=wpsb, in_=wallp[:, :])
        bn7sb = consts.tile([10, 2], F32)
        nc.sync.dma_start(out=bn7sb, in_=bn7[:, :])
        ident = consts.tile([10, 10], F32)
        make_identity(nc, ident)

        # DoubleRow weight views: [128, (tap), (kc), (mc), 128]
        def wview(layer):
            n = 9 * KC[layer] * MC[layer] * 128
            return wsb[:, _OFF[layer] : _OFF[layer] + n].rearrange(
                "p (t k m q) -> p t k m q",
                t=9, k=KC[layer], m=MC[layer], q=128,
            )

        wl4, wl5, wl6 = wview(4), wview(5), wview(6)
        # tap-pair weights: L2 pairs [3,2,128] @0, L2 singles [3,128] @768,
        # L3 pairs [3,2,2,128] @1152, L3 singles [3,2,128] @2688
        w2p = wpsb[:, 0:768].rearrange("p (d j q) -> p d j q", d=3, j=2, q=128)
        w2s = wpsb[:, 768:1152].rearrange("p (d q) -> p d q", d=3, q=128)
        w3p = wpsb[:, 1152:2688].rearrange(
            "p (d j m q) -> p d j m q", d=3, j=2, m=2, q=128
        )
        w3s = wpsb[:, 2688:3456].rearrange(
            "p (d m q) -> p d m q", d=3, m=2, q=128
        )

        # persistent activation buffers (one group's worth, reused)
        # s2d/s3d: copy j=0 is the padded sign grid; copy j=1 is the same
        # data shifted left by one column (B[o] = A[o+1]) so a DoubleRow
        # matmul pairs taps (dy,0)+(dy,1) with a single 16B-aligned stride.
        span2 = g * 1156 + 96
        span3 = g * 324 + 48
        s2d = sbufs.tile([128, 2, span2], FP8)
        s3d = sbufs.tile([128, 2, span3], FP8)
        s2i = s2d[:, :, : g * 1156].rearrange(
            "p k (i y x) -> p k i y x", i=g, y=34, x=34
        )
        s3i = s3d[:, :, : g * 324].rearrange(
            "p k (i y x) -> p k i y x", i=g, y=18, x=18
        )
        s4f = sbufs.tile([128, 2, span4], FP8)
        s5f = sbufs.tile([128, 2, span5], FP8)
        s6f = sbufs.tile([128, 4, span6], FP8)
        s7 = sbufs.tile([128, 4, g, 4, 4], FP8)
        s4i = s4f[:, :, : g * 324].rearrange(
            "p k (i y x) -> p k i y x", i=g, y=18, x=18
        )
        s5i = s5f[:, :, : g * 100].rearrange(
            "p k (i y x) -> p k i y x", i=g, y=10, x=10
        )
        s6i = s6f[:, :, : g * 100].rearrange(
            "p k (i y x) -> p k i y x", i=g, y=10, x=10
        )
        for t in (s2d, s3d, s4f, s5f, s6f):
            nc.gpsimd.memset(t, 1.0)

        def w8(layer, tap, kc, mc):
            o = _woff(layer, tap, kc, mc)
            return wsb[:, o : o + 128]

        def tbias(layer, mc):
            c = CVCOL[layer] + mc
            return cv[:, c : c + 1]

        for grp in range(nb // g):
            i00 = grp * g

            # ------------- L1: conv1 (fp32, 4x row-tiled) + bias/relu/BN1/sign
            # 4 concurrent 32-row PE strips, each on its own 450-pos chunk.
            for ch in range(g // c4):
                xt = xpool.tile([128, 2, 450], F32, tag="xt")
                base = (i00 + ch * c4) * 900
                for st in range(4):
                    nc.sync.dma_start(
                        out=xt[32 * st : 32 * st + 27, :, :].rearrange(
                            "p a b -> p (a b)"
                        ),
                        in_=x1[:, base + st * 900 : base + (st + 1) * 900],
                    )
                for sc in range(2 * c4):
                    st, half = sc % 4, sc // 4
                    p = psum.tile([128, 15, 30], F32, tag=f"ps{sc % 3}", name=f"ps{sc % 3}")
                    nc.tensor.matmul(
                        p, w1sb[32 * st : 32 * st + 27, :],
                        xt[32 * st : 32 * st + 27, half, :],
                        start=True, stop=True, tile_position=(32 * st, 0),
                    )
                    img = ch * c4 + st
                    r0 = half * 15
                    nc.scalar.activation(
                        s2i[:, 0, img, 2 + r0 : 17 + r0, 2:32], p, Sign,
                        bias=tbias(1, 0), scale=1.0,
                    )
                    nc.scalar.activation(
                        s2i[:, 1, img, 2 + r0 : 17 + r0, 1:31], p, Sign,
                        bias=tbias(1, 0), scale=1.0,
                    )

            # ------------- L2: binconv 128->128 (tap-pair DR), pool, BN2, sign
            for img in range(g):
                for rc in range(2):
                    p = psum.tile([128, 16, 32], F32, tag=f"ps{(2 * img + rc) % 3}")
                    k = 0
                    for dy in range(3):
                        r = rc * 16 + dy
                        nc.tensor.matmul(
                            p, w2p[:, dy, :, :],
                            s2i[:, :, img, r : r + 16, 0:32],
                            start=(k == 0), stop=False, perf_mode=DR,
                        )
                        k += 1
                        nc.tensor.matmul(
                            p, w2s[:, dy, :],
                            s2i[:, 0, img, r : r + 16, 2:34],
                            start=False, stop=(dy == 2),
                        )
                        k += 1
                    t1 = post.tile([128, 16, 16], F32, tag="t1")
                    nc.vector.reduce_max(
                        t1, p.rearrange("p y (x two) -> p y x two", two=2), axis=AX
                    )
                    t2 = post.tile([128, 8, 16], F32, tag="t2")
                    nc.vector.reduce_max(
                        t2, t1.rearrange("p (y two) x -> p y x two", two=2), axis=AX
                    )
                    nc.scalar.activation(
                        s3i[:, 0, img, 1 + rc * 8 : 9 + rc * 8, 1:17], t2, Sign,
                        bias=tbias(2, 0), scale=1.0,
                    )
                    nc.scalar.activation(
                        s3i[:, 1, img, 1 + rc * 8 : 9 + rc * 8, 0:16], t2, Sign,
                        bias=tbias(2, 0), scale=1.0,
                    )

            # ------------- L3: binconv 128->256 (tap-pair DR), BN3, pad, sign
            for img in range(g):
                for mc in range(2):
                    p = psum.tile([128, 16, 16], F32, tag=f"ps{(2 * img + mc) % 3}")
                    for dy in range(3):
                        nc.tensor.matmul(
                            p, w3p[:, dy, :, mc, :],
                            s3i[:, :, img, dy : dy + 16, 0:16],
                            start=(dy == 0), stop=False, perf_mode=DR,
                        )
                        nc.tensor.matmul(
                            p, w3s[:, dy, mc, :],
                            s3i[:, 0, img, dy : dy + 16, 2:18],
                            start=False, stop=(dy == 2),
                        )
                    nc.scalar.activation(
                        s4i[:, mc, img, 1:17, 1:17], p, Sign,
                        bias=tbias(3, mc), scale=1.0,
                    )

            # ------------- L4: binconv 256->256 (DoubleRow), pool, BN4, sign
            # weight-reuse: each (tap, mc) weight load feeds 3 psum accumulators
            for mc in range(2):
                for b0 in range(0, g, 3):
                    bs = min(3, g - b0)
                    pss = [
                        psum.tile([128, 16, 16], F32, tag=f"ps{j}", name=f"ps{j}")
                        for j in range(bs)
                    ]
                    for t, (dy, dx) in enumerate(TAPS9):
                        for j in range(bs):
                            nc.tensor.matmul(
                                pss[j], wl4[:, t, 0:2, mc, :],
                                s4i[:, :, b0 + j, dy : dy + 16, dx : dx + 16],
                                start=(t == 0), stop=(t == 8), perf_mode=DR,
                            )
                    for j in range(bs):
                        pv = pss[j]
                        t1 = post.tile([128, 16, 8], F32, tag="t1")
                        nc.vector.reduce_max(
                            t1,
                            pv.rearrange("p y (x two) -> p y x two", two=2),
                            axis=AX,
                        )
                        t2 = post.tile([128, 8, 8], F32, tag="t2")
                        nc.vector.reduce_max(
                            t2, t1.rearrange("p (y two) x -> p y x two", two=2),
                            axis=AX,
                        )
                        nc.scalar.activation(
                            s5i[:, mc, b0 + j, 1:9, 1:9], t2, Sign,
                            bias=tbias(4, mc), scale=1.0,
                        )

            # ------------- L5: binconv 256->512 (DoubleRow), BN5, pad, sign
            nchk = g // cL
            for mc in range(4):
                for cb in range(0, nchk, 3):
                    bs = min(3, nchk - cb)
                    pss = [
                        psum.tile([128, cL * 100], F32, tag=f"ps{j}", name=f"ps{j}")
                        for j in range(bs)
                    ]
                    for t, (dy, dx) in enumerate(TAPS9):
                        for j in range(bs):
                            o = (cb + j) * cL * 100 + dy * 10 + dx
                            nc.tensor.matmul(
                                pss[j], wl5[:, t, 0:2, mc, :],
                                s5f[:, :, o : o + cL * 100],
                                start=(t == 0), stop=(t == 8), perf_mode=DR,
                            )
                    for j in range(bs):
                        i0 = (cb + j) * cL
                        pv = pss[j].rearrange(
                            "p (i y x) -> p i y x", i=cL, y=10, x=10
                        )
                        nc.scalar.activation(
                            s6i[:, mc, i0 : i0 + cL, 1:9, 1:9],
                            pv[:, :, 0:8, 0:8],
                            Sign, bias=tbias(5, mc), scale=1.0,
                        )

            # ------------- L6: binconv 512->512 (DoubleRow), pool, BN6, sign
            for mc in range(4):
                for cb in range(0, nchk, 3):
                    bs = min(3, nchk - cb)
                    pss = [
                        psum.tile([128, cL * 100], F32, tag=f"ps{j}", name=f"ps{j}")
                        for j in range(bs)
                    ]
                    k = 0
                    for kp in range(2):
                        for t, (dy, dx) in enumerate(TAPS9):
                            for j in range(bs):
                                o = (cb + j) * cL * 100 + dy * 10 + dx
                                nc.tensor.matmul(
                                    pss[j], wl6[:, t, 2 * kp : 2 * kp + 2, mc, :],
                                    s6f[:, 2 * kp : 2 * kp + 2, o : o + cL * 100],
                                    start=(k == 0), stop=(k == 17), perf_mode=DR,
                                )
                            k += 1
                    for j in range(bs):
                        i0 = (cb + j) * cL
                        pv = pss[j].rearrange(
                            "p (i y x) -> p i y x", i=cL, y=10, x=10
                        )
                        t1 = post.tile([128, cL, 8, 4], F32, tag="t1")
                        nc.vector.reduce_max(
                            t1,
                            pv[:, :, 0:8, 0:8].rearrange(
                                "p i y (x two) -> p i y x two", two=2
                            ),
                            axis=AX,
                        )
                        t2 = post.tile([128, cL, 4, 4], F32, tag="t2")
                        nc.vector.reduce_max(
                            t2, t1.rearrange("p i (y two) x -> p i y x two", two=2),
                            axis=AX,
                        )
                        nc.scalar.activation(
                            s7[:, mc, i0 : i0 + cL, :, :], t2, Sign,
                            bias=tbias(6, mc), scale=1.0,
                        )

            # ------------- L7: binconv 512->10 (4x4), relu, BN7, softmax
            s7v = s7.rearrange("p k i y x -> p k i (y x)")
            p7 = psum7.tile([10, g], F32, tag="p7")
            k = 0
            for t in range(16):
                for kc in range(4):
                    o = _OFF[7] + (t * 4 + kc) * 10
                    nc.tensor.matmul(
                        p7, wsb[:, o : o + 10], s7v[:, kc, :, t],
                        start=(k == 0), stop=(k == 63),
                    )
                    k += 1
            h7 = post.tile([10, g], F32, tag="h7")
            nc.vector.tensor_scalar_max(h7, p7, 0.0)
            v7 = post.tile([10, g], F32, tag="v7")
            nc.scalar.activation(
                v7, h7, Identity, bias=bn7sb[:, 1:2], scale=bn7sb[:, 0:1]
            )
            pt = psum7.tile([g, 10], F32, tag="pt")
            nc.tensor.transpose(pt, v7, ident)
            mx = post.tile([g, 1], F32, tag="mx")
            nc.vector.reduce_max(mx, pt, axis=AX)
            nmx = post.tile([g, 1], F32, tag="nmx")
            nc.vector.tensor_scalar_mul(nmx, mx, -1.0)
            ex = post.tile([g, 10], F32, tag="ex")
            nc.scalar.activation(ex, pt, Exp, bias=nmx, scale=1.0)
            sm = post.tile([g, 1], F32, tag="sm")
            nc.vector.reduce_sum(sm, ex, axis=AX)
            ri = post.tile([g, 1], F32, tag="ri")
            nc.vector.reciprocal(ri, sm)
            yo = post.tile([g, 10], F32, tag="yo")
            nc.vector.tensor_scalar_mul(yo, ex, ri)
            nc.sync.dma_start(out=y[i00 : i00 + g, :], in_=yo)

    nc.compile()
    return nc


# ------------------------------------------------------------------ host prep

def _thresh_bias(gm, be, m, v):
    """bias such that next-layer input = Sign(pre_bn_value + bias)."""
    a = gm.astype(np.float64) / np.sqrt(v.astype(np.float64) + EPS)
    c = be.astype(np.float64) - a * m.astype(np.float64)
    return np.where(c < 0.0, c / a, BIG).astype(np.float32)  # -T = c/a


def _pack_w(wl):
    """sign(w) [3,3,Cin,Cout] -> [128, 9*KC*MC*128] fp8, (tap,kc,mc,q) order."""
    s = np.where(wl >= 0, 1.0, -1.0).astype(np.float32)
    _, _, cin, cout = wl.shape
    kc, mcn = cin // 128, cout // 128
    a = s.reshape(3, 3, kc, 128, mcn, 128)
    a = np.ascontiguousarray(a.transpose(3, 0, 1, 2, 4, 5))
    return a.reshape(128, 9 * kc * mcn * 128).astype(NP8)


def _prep_shared(inputs):
    d = {k: np.asarray(v, np.float32) for k, v in inputs.items()}

    wall = np.empty((128, WTOT), dtype=NP8)
    for layer in (2, 3, 4, 5, 6):
        wl = _pack_w(d[f"w{layer}"])
        wall[:, _OFF[layer] : _OFF[layer] + wl.shape[1]] = wl
    s7w = np.where(d["w7"] >= 0, 1.0, -1.0).astype(np.float32)
    a = s7w.reshape(4, 4, 4, 128, 10).transpose(3, 0, 1, 2, 4)
    wall[:, _OFF[7] :] = np.ascontiguousarray(a).reshape(128, 640).astype(NP8)

    cvec = np.zeros((128, 14), dtype=np.float32)
    tb1 = _thresh_bias(d["g1"], d["be1"], d["m1"], d["v1"])
    cvec[:, 0] = (d["b1"].astype(np.float64) + tb1.astype(np.float64)).astype(
        np.float32
    )
    for layer in (2, 3, 4, 5, 6):
        tb = _thresh_bias(
            d[f"g{layer}"], d[f"be{layer}"], d[f"m{layer}"], d[f"v{layer}"]
        )
        cvec[:, CVCOL[layer] : CVCOL[layer] + MC[layer]] = tb.reshape(
            MC[layer], 128
        ).T

    a7 = d["g7"].astype(np.float64) / np.sqrt(d["v7"].astype(np.float64) + EPS)
    c7 = d["be7"].astype(np.float64) - a7 * d["m7"].astype(np.float64)
    bn7 = np.stack([a7.astype(np.float32), c7.astype(np.float32)], axis=1)

    wp = np.empty((128, 3456), dtype=NP8)
    s2w = np.where(d["w2"] >= 0, 1.0, -1.0).astype(np.float32)
    s3w = np.where(d["w3"] >= 0, 1.0, -1.0).astype(np.float32)
    for dy in range(3):
        for j in range(2):
            wp[:, (dy * 2 + j) * 128 : (dy * 2 + j + 1) * 128] = s2w[dy, j].astype(NP8)
        wp[:, 768 + dy * 128 : 768 + (dy + 1) * 128] = s2w[dy, 2].astype(NP8)
        for j in range(2):
            for m in range(2):
                o = 1152 + ((dy * 2 + j) * 2 + m) * 128
                wp[:, o : o + 128] = s3w[dy, j, :, m * 128 : (m + 1) * 128].astype(NP8)
        for m in range(2):
            o = 2688 + (dy * 2 + m) * 128
            wp[:, o : o + 128] = s3w[dy, 2, :, m * 128 : (m + 1) * 128].astype(NP8)

    w1r = np.zeros((128, 128), dtype=np.float32)
    for st in range(4):
        w1r[32 * st : 32 * st + 27, :] = d["w1"].reshape(27, 128)
    return d, wall, wp, cvec, bn7, w1r


def _im2col(x):
    """x [B,32,32,3] -> [27, B, 900] f32, row order (dy,dx,c)."""
    from numpy.lib.stride_tricks import sliding_window_view

    sw = sliding_window_view(x, (3, 3), axis=(1, 2))  # [B,30,30,3,3,3]
    im = sw.transpose(4, 5, 3, 0, 1, 2).reshape(27, x.shape[0], 900)
    return np.ascontiguousarray(im)


LAST_RESULTS = None


def kernel(**inputs):
    global LAST_RESULTS
    nb, g = NB, 16
    key = (nb, g)
    if key not in _prog_cache:
        _prog_cache[key] = build_program(nb, g)
    nc = _prog_cache[key]

    d, wall, wp, cvec, bn7, w1r = _prep_shared(inputs)
    im = _im2col(d["x"])

    in_maps = []
    for c in range(NCORES):
        xi = np.ascontiguousarray(im[:, c * nb : (c + 1) * nb, :]).reshape(
            27, nb * 900
        )
        in_maps.append(
            {"x1": xi, "w1": w1r, "wall": wall, "wallp": wp, "cvec": cvec,
             "bn7": bn7}
        )

    trace = bool(int(os.environ.get("KERNEL_TRACE", "0")))
    res = run_bass_kernel_spmd(
        nc, in_maps, core_ids=list(range(NCORES)), trace=trace
    )
    LAST_RESULTS = res
    out = np.concatenate([res.results[i]["y"] for i in range(NCORES)], axis=0)
    return out.astype(np.float32)
